# revision 1
# baseline (speedup 1.0000x reference)
"""Trainium2 Bass kernel for causal multi-head attention (B=4, T=2048, D=1024, H=16).

Sharding: tensor-parallel over heads. Each of the 8 NeuronCores owns 2 heads:
it computes Q/K/V projections for its head-slice over all tokens, runs causal
attention, then an AllToAll re-shards the attention output from head-sharded
to token-sharded so each core computes the final FC layer for its 1024-token
block with the full weight matrix. No reduction collective is needed.

All matmuls run as float32r (TF32-like, full PE rate at moving dim >= 256);
everything else stays fp32. Scores are computed transposed (S^T = K Q^T) so
softmax normalization lands on the PV matmul's free dim; the denominator is
obtained by augmenting V with a ones column, and its reciprocal is broadcast
across partitions with a selector matmul (partition-shifted DVE/DMA ops are
avoided entirely -- they were observed to misbehave on this stack).
"""
import sys

for _p in ("/opt/trn_rl_repo",):
    if _p not in sys.path:
        sys.path.insert(0, _p)

import numpy as np

import concourse.bass as bass
import concourse.mybir as mybir
import concourse.tile as tile
from concourse import bacc
from concourse.bass_utils import run_bass_kernel_spmd

f32 = mybir.dt.float32
f32r = mybir.dt.float32r
bf16 = mybir.dt.bfloat16
EXP = mybir.ActivationFunctionType.Exp

B, T, D, H, HD = 4, 2048, 1024, 16, 64
NCORES = 8
HPC = H // NCORES          # heads per core = 2
BT = B * T                 # 8192
CH = 512                   # token chunk (matmul moving dim)
NCH_B = T // CH            # 4 projection chunks per batch
QC = T // CH               # 4 query chunks per batch
NKV_B = T // 128           # 16 kv tiles of 128 per batch
ROWS = BT // NCORES        # 1024 output token rows per core
SCALE = 1.0 / 8.0          # 1/sqrt(HD)

_CACHE = {}


def _build(sim=False, no_collective=False, reps=1, n_ag=1, phases=('qkv','attn','fc'), dup=()):
    nc = bacc.Bacc("TRN2", target_bir_lowering=False, debug=False,
                   num_devices=1 if sim else NCORES)

    xT = nc.dram_tensor("xT", [D, BT], bf16, kind="ExternalInput").ap()
    wqkv = nc.dram_tensor("wqkv", [D, 3 * 128], bf16, kind="ExternalInput").ap()
    bqkv = nc.dram_tensor("bqkv", [1, 3 * 128], f32, kind="ExternalInput").ap()
    wfc_d = nc.dram_tensor("wfc", [D, 128], f32, kind="ExternalInput").ap()
    bfc_d = nc.dram_tensor("bfc", [1, 128], f32, kind="ExternalInput").ap()
    hm_d = nc.dram_tensor("hm", [128, 896], f32, kind="ExternalInput").ap()
    zl_d = nc.dram_tensor("zl", [65, 64], f32, kind="ExternalInput").ap()
    id_d = nc.dram_tensor("ident", [128, 128], f32, kind="ExternalInput").ap()
    ones_d = nc.dram_tensor("ones", [128, CH], f32, kind="ExternalInput").ap()
    zer_d = nc.dram_tensor("zer", [65, CH], f32, kind="ExternalInput").ap()
    outT = nc.dram_tensor("outT", [128, BT], f32, kind="ExternalOutput").ap()

    with tile.TileContext(nc) as tc:
        with tc.tile_pool(name="const", bufs=1) as cst, \
             tc.tile_pool(name="dram", bufs=1, space="DRAM") as dpool:

            # ---- constants (host-provided) ----
            ones_r = cst.tile([128, CH], f32r)
            nc.sync.dma_start(ones_r[:], ones_d[:].bitcast(f32r))
            hm = cst.tile([128, 896], f32)
            nc.sync.dma_start(hm[:], hm_d[:])
            zl = cst.tile([65, 64], f32r)       # selector: row 64 = 1, rest 0
            nc.sync.dma_start(zl[:], zl_d[:].bitcast(f32r))
            bias_q = cst.tile([1, 3 * 128], f32r)
            nc.sync.dma_start(bias_q[:], bqkv[:].bitcast(f32r))
            bias_f = cst.tile([1, 128], f32r)
            nc.sync.dma_start(bias_f[:], bfc_d[:].bitcast(f32r))
            # reciprocal staging tiles (rows 0..63 stay zero forever)
            rc = []
            for h in range(HPC):
                t = cst.tile([65, CH], f32r, name=f"recip{h}")
                nc.sync.dma_start(t[:], zer_d[:].bitcast(f32r))
                rc.append(t)
            rtmp = cst.tile([65, CH], f32)      # fp32 reciprocal staging
            ones_b = cst.tile([1, CH], bf16)
            nc.vector.tensor_copy(ones_b[:], ones_r[0:1, :].bitcast(f32))
            biasb = cst.tile([1, 3 * 128], bf16)
            nc.vector.tensor_copy(biasb[:], bias_q[:].bitcast(f32))

            # attention output, head-sharded: per local head [64, BT]
            attn = [cst.tile([64, BT], f32r, name=f"attn{h}")
                    for h in range(HPC)]

            # qkv weights: 8 d-tiles of [128, 384] = [q128 | k128 | v128]
            wq = cst.tile([128, 8 * 384], bf16)
            for d in range(8):
                nc.sync.dma_start(wq[:, d * 384:(d + 1) * 384],
                                  wqkv[d * 128:(d + 1) * 128, :])

            for _rep in range(reps):
                # ---- per-batch QKV projection + attention ----
                with tc.tile_pool(name="work", bufs=1) as wk, \
                     tc.tile_pool(name="ps", bufs=1, space="PSUM") as ps:
                    for b in range(B):
                        t0 = b * T
                        qt = wk.tile([128, T], f32r, tag="qt", bufs=2, name=f"qt{b}")
                        kt = wk.tile([128, T], f32r, tag="kt", bufs=2, name=f"kt{b}")
                        vsb = wk.tile([128, NKV_B * 130], f32r, tag="vsb", bufs=2,
                                      name=f"vsb{b}")
                        # ones columns (denominator) for all 16 kv tiles of batch b
                        v3 = vsb.rearrange("p (t c) -> p t c", c=130)
                        src1 = ones_d[:, 0:NKV_B].rearrange("p (t c) -> p t c", c=1)
                        nc.sync.dma_start(v3[:, :, 64:65], src1.bitcast(f32r))
                        nc.sync.dma_start(v3[:, :, 129:130], src1.bitcast(f32r))

                        for ch in [c for c in range(NCH_B) for _ in range(2 if 'qkv' in dup else 1)]:
                            c0 = t0 + ch * CH
                            xt = wk.tile([128, 8 * CH], bf16, tag="xt", bufs=2,
                                         name=f"xt{b}_{ch}")
                            xt3 = xt.rearrange("p (d c) -> p d c", d=8)
                            xs3 = xT[:, c0:c0 + CH].rearrange(
                                "(d p) c -> p d c", p=128)
                            nc.sync.dma_start(xt3[:], xs3)
                            cs = ch * CH
                            # Q^T chunk
                            psq = ps.tile([128, CH], f32, tag="mm", bufs=2,
                                          name=f"psq{b}_{ch}")
                            for d in range(8):
                                nc.tensor.matmul(psq[:],
                                                 wq[:, d * 384:d * 384 + 128],
                                                 xt[:, d * CH:(d + 1) * CH],
                                                 start=(d == 0), stop=False)
                            nc.tensor.matmul(psq[:], bias_q[0:1, 0:128],
                                             ones_r[0:1, :], start=False, stop=True)
                            nc.vector.tensor_copy(qt[:, cs:cs + CH], psq[:])
                            # K^T chunk
                            psk = ps.tile([128, CH], f32, tag="mm", bufs=2,
                                          name=f"psk{b}_{ch}")
                            for d in range(8):
                                nc.tensor.matmul(psk[:],
                                                 wq[:, d * 384 + 128:d * 384 + 256],
                                                 xt[:, d * CH:(d + 1) * CH],
                                                 start=(d == 0), stop=False)
                            nc.tensor.matmul(psk[:], bias_q[0:1, 128:256],
                                             ones_r[0:1, :], start=False, stop=True)
                            nc.vector.tensor_copy(kt[:, cs:cs + CH], psk[:])
                            # V directly token-major: lhsT = x tile, rhs = W_v
                            for sb in range(CH // 128):
                                kvt = ch * 4 + sb   # kv tile idx within batch
                                psv = ps.tile([128, 128], f32, tag="mm", bufs=2,
                                              name=f"psv{b}_{ch}_{sb}")
                                for d in range(8):
                                    nc.tensor.matmul(
                                        psv[:],
                                        xt3[:, d, sb * 128:(sb + 1) * 128],
                                        wq[:, d * 384 + 256:d * 384 + 384],
                                        start=(d == 0), stop=False)
                                nc.tensor.matmul(psv[:], ones_b[0:1, 0:128],
                                                 biasb[0:1, 256:384],
                                                 start=False, stop=True)
                                base = kvt * 130
                                nc.vector.tensor_copy(vsb[:, base:base + 64],
                                                      psv[:, 0:64])
                                nc.vector.tensor_copy(vsb[:, base + 65:base + 129],
                                                      psv[:, 64:128])

                        # ---- causal attention for batch b ----
                        for qc in range(QC):
                            g0 = t0 + qc * CH
                            nkv = 4 * (qc + 1)
                            pv = [ps.tile([128, CH], f32, tag=f"pv{h}", bufs=1,
                                          name=f"pv{h}_{b}_{qc}")
                                  for h in range(HPC)]
                            for ki in range(nkv):
                                diag = ki - 4 * qc  # >=0 on diagonal block tiles
                                st = ps.tile([128, 2 * CH], f32, tag="s", bufs=2,
                                             name=f"s_{b}_{qc}_{ki}")
                                pt = wk.tile([128, 2 * CH], f32r, tag="p", bufs=3,
                                             name=f"p_{b}_{qc}_{ki}")
                                for h in range(HPC):
                                    nc.tensor.matmul(
                                        st[:, h * CH:(h + 1) * CH],
                                        kt[64 * h:64 * h + 64,
                                           ki * 128:(ki + 1) * 128],
                                        qt[64 * h:64 * h + 64,
                                           qc * CH:(qc + 1) * CH],
                                        start=True, stop=True,
                                        tile_position=(64 * h, 0))
                                nc.scalar.activation(pt[:], st[:], EXP,
                                                     scale=SCALE)
                                if diag >= 0:
                                    off = 384 - 128 * diag
                                    for h in range(HPC):
                                        nc.vector.tensor_mul(
                                            pt[:, h * CH:(h + 1) * CH],
                                            pt[:, h * CH:(h + 1) * CH],
                                            hm[:, off:off + CH])
                                for h in range(HPC):
                                    vb = ki * 130 + 65 * h
                                    nc.tensor.matmul(pv[h][0:65, :],
                                                     vsb[:, vb:vb + 65],
                                                     pt[:, h * CH:(h + 1) * CH],
                                                     start=(ki == 0),
                                                     stop=(ki == nkv - 1))
                            # normalize: reciprocal of denom row, broadcast, mul
                            for h in range(HPC):
                                nc.vector.reciprocal(rtmp[64:65, :],
                                                     pv[h][64:65, :])
                                nc.vector.tensor_copy(rc[h][64:65, :],
                                                      rtmp[64:65, :])
                                bc = ps.tile([64, CH], f32, tag="mm", bufs=2,
                                             name=f"bc{h}_{b}_{qc}")
                                nc.tensor.matmul(bc[:], zl[:], rc[h][:],
                                                 start=True, stop=True)
                                rb = wk.tile([64, CH], f32, tag="rb", bufs=2,
                                             name=f"rb{h}_{b}_{qc}")
                                nc.vector.tensor_copy(rb[:], bc[:])
                                nc.vector.tensor_mul(attn[h][:, g0:g0 + CH],
                                                     pv[h][0:64, :], rb[:])

                # ---- per-batch AllGather (overlaps later batches) ----
                ag_outs = []
                for b in range(B):
                    t0 = b * T
                    ag_in = dpool.tile([128, T], f32, name=f"ag_in{b}")
                    ag_out = dpool.tile([NCORES * 128, T], f32,
                                        name=f"ag_out{b}")
                    for h in range(HPC):
                        nc.sync.dma_start(
                            ag_in[64 * h:64 * h + 64, :],
                            attn[h][:, t0:t0 + T].bitcast(f32))
                    if sim or no_collective:
                        nc.sync.dma_start(ag_out[0:128, :], ag_in[:])
                    else:
                        for _agi in range(n_ag):
                            nc.gpsimd.collective_compute(
                                "AllGather", mybir.AluOpType.bypass,
                                replica_groups=[list(range(NCORES))],
                                ins=[ag_in.opt()], outs=[ag_out.opt()])
                    ag_outs.append(ag_out)

                # ---- final FC: this core computes its 128 output features for
                # all tokens (weight slice is per-core host input) ----
                with tc.tile_pool(name="fcp", bufs=1) as fcp, \
                     tc.tile_pool(name="psc", bufs=1, space="PSUM") as psc:
                    wfc = fcp.tile([128, 8 * 128], f32r)
                    for d in range(8):
                        nc.sync.dma_start(
                            wfc[:, d * 128:(d + 1) * 128],
                            wfc_d[d * 128:(d + 1) * 128, :].bitcast(f32r))
                    for oc in [o for o in range(BT // CH if 'fc' in phases else 0) for _ in range(2 if 'fc' in dup else 1)]:
                        fci = fcp.tile([128, 8 * CH], f32r, tag="fci", bufs=3,
                                       name=f"fci{oc}")
                        agb = ag_outs[oc // QC]
                        lc = oc % QC
                        for d in range(8):
                            nc.sync.dma_start(
                                fci[:, d * CH:(d + 1) * CH],
                                agb[d * 128:(d + 1) * 128,
                                    lc * CH:(lc + 1) * CH].bitcast(f32r))
                        pfc = psc.tile([128, CH], f32, tag="fc", bufs=4,
                                       name=f"pfc{oc}")
                        for d in range(8):
                            nc.tensor.matmul(pfc[:],
                                             wfc[:, d * 128:(d + 1) * 128],
                                             fci[:, d * CH:(d + 1) * CH],
                                             start=(d == 0), stop=False)
                        nc.tensor.matmul(pfc[:], bias_f[0:1, :],
                                         ones_r[0:1, :], start=False, stop=True)
                        ost = fcp.tile([128, CH], f32, tag="ost", bufs=4,
                                       name=f"ost{oc}")
                        nc.vector.tensor_copy(ost[:], pfc[:])
                        nc.sync.dma_start(outT[:, oc * CH:(oc + 1) * CH], ost[:])

    nc.compile()
    return nc


def _host_inputs(x, W_qkv, b_qkv, W_fc, b_fc):
    import ml_dtypes
    x = np.asarray(x, dtype=np.float32)
    W_qkv = np.asarray(W_qkv, dtype=np.float32)
    b_qkv = np.asarray(b_qkv, dtype=np.float32)
    W_fc = np.asarray(W_fc, dtype=np.float32)
    b_fc = np.asarray(b_fc, dtype=np.float32)

    xT = np.ascontiguousarray(x.reshape(BT, D).T).astype(ml_dtypes.bfloat16)
    hm = (np.arange(128)[:, None]
          <= np.arange(896)[None, :] - 384).astype(np.float32)
    zl = np.zeros((65, 64), np.float32)
    zl[64, :] = 1.0
    ident = np.eye(128, dtype=np.float32)
    ones = np.ones((128, CH), np.float32)
    zer = np.zeros((65, CH), np.float32)
    in_maps = []
    for c in range(NCORES):
        f0 = c * (HPC * HD)  # 128*c
        wfc_c = np.ascontiguousarray(W_fc[:, f0:f0 + 128])
        bfc_c = np.ascontiguousarray(b_fc[None, f0:f0 + 128])
        wq_c = np.ascontiguousarray(np.concatenate(
            [W_qkv[:, p * D + f0: p * D + f0 + 128] for p in range(3)],
            axis=1).astype(ml_dtypes.bfloat16))
        bq_c = np.ascontiguousarray(np.concatenate(
            [b_qkv[p * D + f0: p * D + f0 + 128] for p in range(3)])[None, :])
        in_maps.append({
            "xT": xT, "wqkv": wq_c, "bqkv": bq_c, "wfc": wfc_c, "bfc": bfc_c,
            "hm": hm, "zl": zl, "ident": ident, "ones": ones, "zer": zer,
        })
    return in_maps


def _get_nc():
    if "nc" not in _CACHE:
        _CACHE["nc"] = _build()
    return _CACHE["nc"]


def _assemble(results):
    blocks = [results[c]["outT"] for c in range(NCORES)]
    full = np.concatenate(blocks, axis=0)          # [D, BT], feature-major
    return np.ascontiguousarray(full.T).reshape(B, T, D).astype(np.float32)


def kernel(x, W_qkv, b_qkv, W_fc, b_fc):
    nc = _get_nc()
    in_maps = _host_inputs(x, W_qkv, b_qkv, W_fc, b_fc)
    res = run_bass_kernel_spmd(nc, in_maps, list(range(NCORES)))
    return _assemble(res.results)



# revision 5
# speedup vs baseline: 1.4626x; 1.4626x over previous
"""Trainium2 Bass kernel for causal multi-head attention (B=4, T=2048, D=1024, H=16).

Sharding: tensor-parallel over heads for QKV+attention (each of 8 cores owns
2 heads over all tokens), then an AllToAll re-shards from head-sharded to
token-sharded so each core computes the final FC over the full feature dim
for its 256-token slice of each batch.

All matmuls run in bf16 (fp32 streams at half PE rate; bf16 at full), with
fp32 PSUM accumulation. Scores are computed transposed (S^T = K Q^T, two
heads packed in PE quadrants via tile_position) so softmax normalization
lands on the PV matmul's free dim; the denominator comes from a ones column
augmented into V. Normalization is deferred to the end of each batch (the
per-chunk broadcast chain caused PE bubbles), and the reciprocal broadcast
across partitions uses a selector matmul (partition-shifted DVE/DMA ops are
avoided - they misbehave on this stack). QKV projection of batch b+1 and the
FC of batch b-1 are interleaved into batch b's attention loop to keep the PE
fed while the scalar engine runs exp.
"""
import sys

for _p in ("/opt/trn_rl_repo",):
    if _p not in sys.path:
        sys.path.insert(0, _p)

import numpy as np

import concourse.bass as bass
import concourse.mybir as mybir
import concourse.tile as tile
from concourse import bacc
from concourse.bass_utils import run_bass_kernel_spmd

f32 = mybir.dt.float32
bf16 = mybir.dt.bfloat16
EXP = mybir.ActivationFunctionType.Exp

B, T, D, H, HD = 4, 2048, 1024, 16, 64
NCORES = 8
HPC = H // NCORES          # heads per core = 2
BT = B * T                 # 8192
CH = 512                   # token chunk (q chunk / projection chunk)
NCH_B = T // CH            # 4 projection chunks per batch
QC = T // CH               # 4 query chunks per batch
NKV_B = T // 128           # 16 kv tiles of 128 per batch
TOK = T // NCORES          # 256 tokens per core per batch (after AllToAll)
SCALE = 1.0 / 8.0          # 1/sqrt(HD)

_CACHE = {}


def _build(no_collective=False):
    nc = bacc.Bacc("TRN2", target_bir_lowering=False, debug=False,
                   num_devices=NCORES)

    xT = nc.dram_tensor("xT", [D, BT], bf16, kind="ExternalInput").ap()
    wq_d = nc.dram_tensor("wq", [128, 8 * 384], bf16, kind="ExternalInput").ap()
    bq_d = nc.dram_tensor("bq", [1, 3 * 128], bf16, kind="ExternalInput").ap()
    wfc_d = nc.dram_tensor("wfc", [128, 64 * 128], bf16,
                           kind="ExternalInput").ap()
    bfc_d = nc.dram_tensor("bfc", [1, D], bf16, kind="ExternalInput").ap()
    hm_d = nc.dram_tensor("hm", [128, 896], bf16, kind="ExternalInput").ap()
    outT = nc.dram_tensor("outT", [D, B * TOK], f32, kind="ExternalOutput").ap()

    with tile.TileContext(nc) as tc:
        with tc.tile_pool(name="const", bufs=1) as cst, \
             tc.tile_pool(name="dram", bufs=1, space="DRAM") as dpool, \
             tc.tile_pool(name="work", bufs=1) as wk, \
             tc.tile_pool(name="ps", bufs=1, space="PSUM") as ps:

            # ---- weights needed first (QKV of batch 0) ----
            wq = cst.tile([128, 8 * 384], bf16)
            nc.sync.dma_start(wq[:], wq_d[:])
            biasq = cst.tile([1, 3 * 128], bf16)
            nc.sync.dma_start(biasq[:], bq_d[:])
            onesb = cst.tile([1, CH], bf16)
            nc.gpsimd.memset(onesb[:], 1.0)
            # selector for reciprocal broadcast: row 64 = 1, rest 0
            zl = cst.tile([65, 64], bf16)
            nc.gpsimd.memset(zl[:], 0.0)
            nc.gpsimd.memset(zl[64:65, :], 1.0)

            # ---- deferred constants (needed later; don't block first mm) ----
            def _late_consts():
                hm = cst.tile([128, 896], bf16)
                nc.sync.dma_start(hm[:], hm_d[:])
                wfc = cst.tile([128, 64 * 128], bf16)
                nc.sync.dma_start(wfc[:], wfc_d[:])
                biasf = cst.tile([1, D], bf16)
                nc.sync.dma_start(biasf[:], bfc_d[:])
                # per-head reciprocal staging: row 64 = recip, rows 0-63
                # zeroed once (garbage would poison the selector matmul)
                rc = []
                for h in range(HPC):
                    t = cst.tile([65, T], bf16, name=f"rc{h}")
                    nc.gpsimd.memset(t[0:64, :], 0.0)
                    rc.append(t)
                return hm, wfc, biasf, rc

            # ---- per-batch state (double buffered across the pipeline) ----
            def alloc_batch(b):
                qt = wk.tile([128, T], bf16, tag="qt", bufs=2, name=f"qt{b}")
                kt = wk.tile([128, T], bf16, tag="kt", bufs=2, name=f"kt{b}")
                vsb = wk.tile([128, NKV_B * 130], bf16, tag="vsb", bufs=2,
                              name=f"vsb{b}")
                v3 = vsb.rearrange("p (t c) -> p t c", c=130)
                nc.gpsimd.memset(v3[:, :, 64:65], 1.0)
                nc.gpsimd.memset(v3[:, :, 129:130], 1.0)
                praw = [wk.tile([64, T], bf16, tag=f"praw{h}", bufs=2,
                                name=f"praw{h}_{b}") for h in range(HPC)]
                return qt, kt, vsb, praw

            def qkv_chunk(b, ch, st):
                """Project one 512-token chunk of batch b into qt/kt/vsb."""
                qt, kt, vsb = st[0], st[1], st[2]
                t0 = b * T
                c0 = t0 + ch * CH
                cs = ch * CH
                xt = wk.tile([128, 8 * CH], bf16, tag="xt", bufs=2,
                             name=f"xt{b}_{ch}")
                xt3 = xt.rearrange("p (d c) -> p d c", d=8)
                xs3 = xT[:, c0:c0 + CH].rearrange("(d p) c -> p d c", p=128)
                nc.sync.dma_start(xt3[:], xs3)
                # Q^T and K^T chunks share one PSUM tile
                psqk = ps.tile([128, 2 * CH], f32, tag="st", bufs=2,
                               name=f"psqk{b}_{ch}")
                for d in range(8):
                    nc.tensor.matmul(psqk[:, 0:CH],
                                     wq[:, d * 384:d * 384 + 128],
                                     xt[:, d * CH:(d + 1) * CH],
                                     start=(d == 0), stop=False)
                nc.tensor.matmul(psqk[:, 0:CH], biasq[0:1, 0:128],
                                 onesb[0:1, :], start=False, stop=True)
                for d in range(8):
                    nc.tensor.matmul(psqk[:, CH:2 * CH],
                                     wq[:, d * 384 + 128:d * 384 + 256],
                                     xt[:, d * CH:(d + 1) * CH],
                                     start=(d == 0), stop=False)
                nc.tensor.matmul(psqk[:, CH:2 * CH], biasq[0:1, 128:256],
                                 onesb[0:1, :], start=False, stop=True)
                nc.vector.tensor_copy(qt[:, cs:cs + CH], psqk[:, 0:CH])
                nc.vector.tensor_copy(kt[:, cs:cs + CH], psqk[:, CH:2 * CH])
                # V token-major: lhsT = x tile, rhs = W_v columns
                psv = ps.tile([128, 2 * CH], f32, tag="st", bufs=2,
                              name=f"psv{b}_{ch}")
                for sb in range(CH // 128):
                    kvt = ch * 4 + sb
                    vo = sb * 128
                    for d in range(8):
                        nc.tensor.matmul(
                            psv[:, vo:vo + 128],
                            xt3[:, d, sb * 128:(sb + 1) * 128],
                            wq[:, d * 384 + 256:d * 384 + 384],
                            start=(d == 0), stop=False)
                    nc.tensor.matmul(psv[:, vo:vo + 128], onesb[0:1, 0:128],
                                     biasq[0:1, 256:384],
                                     start=False, stop=True)
                    base = kvt * 130
                    nc.vector.tensor_copy(vsb[:, base:base + 64],
                                          psv[:, vo:vo + 64])
                    nc.vector.tensor_copy(vsb[:, base + 65:base + 129],
                                          psv[:, vo + 64:vo + 128])

            def attn_qc(b, qc, st, hm, rc):
                """Causal attention for query chunk qc of batch b."""
                qt, kt, vsb, praw = st
                nkv = 4 * (qc + 1)
                pv = [ps.tile([65, CH], f32, tag=f"pv{h}", bufs=1,
                              name=f"pv{h}_{b}_{qc}") for h in range(HPC)]
                for ki in range(nkv):
                    diag = ki - 4 * qc
                    stt = ps.tile([128, 2 * CH], f32, tag="st", bufs=2,
                                  name=f"s_{b}_{qc}_{ki}")
                    pt = wk.tile([128, 2 * CH], bf16, tag="pt", bufs=3,
                                 name=f"p_{b}_{qc}_{ki}")
                    for h in range(HPC):
                        nc.tensor.matmul(
                            stt[:, h * CH:(h + 1) * CH],
                            kt[64 * h:64 * h + 64, ki * 128:(ki + 1) * 128],
                            qt[64 * h:64 * h + 64, qc * CH:(qc + 1) * CH],
                            start=True, stop=True,
                            tile_position=(64 * h, 0))
                    nc.scalar.activation(pt[:], stt[:], EXP, scale=SCALE)
                    if diag >= 0:
                        off = 384 - 128 * diag
                        for h in range(HPC):
                            nc.vector.tensor_mul(
                                pt[:, h * CH:(h + 1) * CH],
                                pt[:, h * CH:(h + 1) * CH],
                                hm[:, off:off + CH])
                    for h in range(HPC):
                        vb = ki * 130 + 65 * h
                        nc.tensor.matmul(pv[h][0:65, :],
                                         vsb[:, vb:vb + 65],
                                         pt[:, h * CH:(h + 1) * CH],
                                         start=(ki == 0),
                                         stop=(ki == nkv - 1))
                # stash raw PV + reciprocal of the denominator row
                for h in range(HPC):
                    nc.vector.tensor_copy(praw[h][:, qc * CH:(qc + 1) * CH],
                                          pv[h][0:64, :])
                    with nc.allow_low_precision(reason="bf16 recip of denom"):
                        nc.vector.reciprocal(
                            rc[h][64:65, qc * CH:(qc + 1) * CH],
                            pv[h][64:65, :])

            def normalize_batch(b, st, rc):
                """praw *= broadcast(1/denom) via selector matmul."""
                praw = st[3]
                for qc in range(QC):
                    for h in range(HPC):
                        bcq = ps.tile([128, CH], f32, tag="aux", bufs=2,
                                      name=f"bc{h}_{b}_{qc}")
                        nc.tensor.matmul(bcq[0:64, :], zl[:, 0:64],
                                         rc[h][:, qc * CH:(qc + 1) * CH],
                                         start=True, stop=True)
                        rbs = wk.tile([64, CH], bf16, tag="rbs", bufs=2,
                                      name=f"rbs{h}_{b}_{qc}")
                        nc.vector.tensor_copy(rbs[:], bcq[0:64, :])
                        nc.vector.tensor_mul(
                            praw[h][:, qc * CH:(qc + 1) * CH],
                            praw[h][:, qc * CH:(qc + 1) * CH], rbs[:])

            def a2a_batch(b, st):
                """Re-shard batch b attention output: head- to token-sharded."""
                praw = st[3]
                ag_in = dpool.tile([NCORES * 128, TOK], bf16,
                                   name=f"ag_in{b}")
                ag_out = dpool.tile([NCORES * 128, TOK], bf16,
                                    name=f"ag_out{b}")
                div = ag_in.rearrange("(d p) c -> p d c", p=128)
                for h in range(HPC):
                    src = praw[h].rearrange("p (d c) -> p d c", c=TOK)
                    nc.sync.dma_start(div[64 * h:64 * h + 64, :, :], src)
                if no_collective:
                    nc.sync.dma_start(ag_out[:], ag_in[:])
                else:
                    nc.gpsimd.collective_compute(
                        "AllToAll", mybir.AluOpType.bypass,
                        replica_groups=[list(range(NCORES))],
                        ins=[ag_in.opt()], outs=[ag_out.opt()])
                return ag_out

            def fc_batch(b, ag_out, wfc, biasf):
                """Full FC for this core's 256-token slice of batch b."""
                fci = wk.tile([128, 8 * TOK], bf16, tag="fci", bufs=2,
                              name=f"fci{b}")
                fci3 = fci.rearrange("p (d c) -> p d c", d=8)
                src = ag_out.rearrange("(d p) c -> p d c", p=128)
                nc.sync.dma_start(fci3[:], src)
                ost = wk.tile([128, 8 * TOK], f32, tag="ost", bufs=2,
                              name=f"ost{b}")
                for fo in range(8):
                    pfc = ps.tile([128, CH], f32, tag="aux", bufs=2,
                                  name=f"pfc{b}_{fo}")
                    for d in range(8):
                        nc.tensor.matmul(
                            pfc[:, 0:TOK],
                            wfc[:, (fo * 8 + d) * 128:(fo * 8 + d + 1) * 128],
                            fci[:, d * TOK:(d + 1) * TOK],
                            start=(d == 0), stop=False)
                    nc.tensor.matmul(pfc[:, 0:TOK],
                                     biasf[0:1, fo * 128:(fo + 1) * 128],
                                     onesb[0:1, 0:TOK], start=False, stop=True)
                    nc.vector.tensor_copy(ost[:, fo * TOK:(fo + 1) * TOK],
                                          pfc[:, 0:TOK])
                dst = outT.rearrange("(f p) c -> p f c", p=128)[
                    :, :, b * TOK:(b + 1) * TOK]
                osrc = ost.rearrange("p (f c) -> p f c", c=TOK)
                nc.sync.dma_start(dst, osrc)

            # ================= schedule =================
            states = [None] * B
            states[0] = alloc_batch(0)
            qkv_chunk(0, 0, states[0])
            hm, wfc, biasf, rc = _late_consts()
            for ch in range(1, NCH_B):
                qkv_chunk(0, ch, states[0])

            ag_outs = [None] * B
            for b in range(B):
                if b + 1 < B:
                    states[b + 1] = alloc_batch(b + 1)
                for qc in range(QC):
                    attn_qc(b, qc, states[b], hm, rc)
                    # keep PE fed during exp: project next batch's chunk
                    if b + 1 < B:
                        qkv_chunk(b + 1, qc, states[b + 1])
                    # FC for the previous batch (its AllToAll is done by now)
                    if qc == 2 and b >= 1:
                        fc_batch(b - 1, ag_outs[b - 1], wfc, biasf)
                normalize_batch(b, states[b], rc)
                ag_outs[b] = a2a_batch(b, states[b])
            fc_batch(B - 1, ag_outs[B - 1], wfc, biasf)

    nc.compile()
    return nc


def _host_inputs(x, W_qkv, b_qkv, W_fc, b_fc):
    import ml_dtypes
    bf = ml_dtypes.bfloat16
    x = np.asarray(x, dtype=np.float32)
    W_qkv = np.asarray(W_qkv, dtype=np.float32)
    b_qkv = np.asarray(b_qkv, dtype=np.float32)
    W_fc = np.asarray(W_fc, dtype=np.float32)
    b_fc = np.asarray(b_fc, dtype=np.float32)

    xT = np.ascontiguousarray(x.reshape(BT, D).T).astype(bf)
    hm = (np.arange(128)[:, None]
          <= np.arange(896)[None, :] - 384).astype(bf)
    # full FC weights prepacked to SBUF layout [p, (f*8+d)*128 + c]
    wfc = np.ascontiguousarray(
        W_fc.reshape(8, 128, 8, 128).transpose(1, 2, 0, 3).reshape(128, 8192)
    ).astype(bf)
    bfc = np.ascontiguousarray(b_fc[None, :]).astype(bf)
    in_maps = []
    for c in range(NCORES):
        f0 = c * (HPC * HD)  # 128*c
        wqs = np.concatenate(
            [W_qkv[:, p * D + f0: p * D + f0 + 128] for p in range(3)],
            axis=1)  # [1024, 384] = [q|k|v]
        wq_c = np.ascontiguousarray(
            wqs.reshape(8, 128, 384).transpose(1, 0, 2).reshape(128, 3072)
        ).astype(bf)
        bq_c = np.ascontiguousarray(np.concatenate(
            [b_qkv[p * D + f0: p * D + f0 + 128] for p in range(3)])[None, :]
        ).astype(bf)
        in_maps.append({
            "xT": xT, "wq": wq_c, "bq": bq_c, "wfc": wfc, "bfc": bfc,
            "hm": hm,
        })
    return in_maps


def _get_nc():
    if "nc" not in _CACHE:
        _CACHE["nc"] = _build()
    return _CACHE["nc"]


def _assemble(results):
    full = np.empty((BT, D), dtype=np.float32)
    for c in range(NCORES):
        o = results[c]["outT"]  # [1024 features, 4*256 tokens]
        for b in range(B):
            full[b * T + c * TOK: b * T + (c + 1) * TOK, :] = \
                o[:, b * TOK:(b + 1) * TOK].T
    return full.reshape(B, T, D)


def kernel(x, W_qkv, b_qkv, W_fc, b_fc):
    nc = _get_nc()
    in_maps = _host_inputs(x, W_qkv, b_qkv, W_fc, b_fc)
    res = run_bass_kernel_spmd(nc, in_maps, list(range(NCORES)))
    return _assemble(res.results)


# revision 9
# speedup vs baseline: 1.4737x; 1.0076x over previous
"""Trainium2 Bass kernel for causal multi-head attention (B=4, T=2048, D=1024, H=16).

Sharding: tensor-parallel over heads for QKV+attention (each of 8 cores owns
2 heads over all tokens), then AllToAll re-shards from head-sharded to
token-sharded so each core computes the final FC over the full feature dim
for its 128-token slice of each half-batch.

All matmuls run in bf16 (fp32 streams at half PE rate; bf16 at full), with
fp32 PSUM accumulation. Scores are computed transposed (S^T = K Q^T, two
heads packed in PE quadrants via tile_position) so softmax normalization
lands on the PV matmul's free dim; the denominator comes from a ones column
augmented into V. Normalization is deferred out of the inner loop (the
per-chunk broadcast chain caused PE bubbles + HAM re-throttling), uses
reciprocal_approx_fast (plain reciprocal is 3.3us for [1,512] and stalls
the next chunk via a PSUM WAR), and broadcasts across partitions with a
selector matmul (partition-shifted DVE/DMA ops misbehave on this stack;
SBUF-side DMA access patterns must keep the partition dim outermost).
The AllToAll runs per half-batch so the last collective+FC tail is short,
and QKV of batch b+1 / FC of batch b-1 interleave into batch b's attention
to keep the PE fed while the scalar engine runs exp.
"""
import sys

for _p in ("/opt/trn_rl_repo",):
    if _p not in sys.path:
        sys.path.insert(0, _p)

import numpy as np

import concourse.bass as bass
import concourse.mybir as mybir
import concourse.tile as tile
from concourse import bacc
from concourse.bass_utils import run_bass_kernel_spmd

f32 = mybir.dt.float32
bf16 = mybir.dt.bfloat16
EXP = mybir.ActivationFunctionType.Exp

B, T, D, H, HD = 4, 2048, 1024, 16, 64
NCORES = 8
HPC = H // NCORES          # heads per core = 2
BT = B * T                 # 8192
CH = 512                   # token chunk (q chunk / projection chunk)
NCH_B = T // CH            # 4 projection chunks per batch
QC = T // CH               # 4 query chunks per batch
NKV_B = T // 128           # 16 kv tiles of 128 per batch
HTOK = 128                 # tokens per core per half-batch (after AllToAll)
NHALF = 2 * B              # 8 half-batches
SCALE = 1.0 / 8.0          # 1/sqrt(HD)

_CACHE = {}


def _build(no_collective=False):
    nc = bacc.Bacc("TRN2", target_bir_lowering=False, debug=False,
                   num_devices=NCORES)

    xT = nc.dram_tensor("xT", [D, BT], bf16, kind="ExternalInput").ap()
    wq_d = nc.dram_tensor("wq", [128, 8 * 384], bf16, kind="ExternalInput").ap()
    bq_d = nc.dram_tensor("bq", [1, 3 * 128], bf16, kind="ExternalInput").ap()
    wfc_d = nc.dram_tensor("wfc", [128, 64 * 128], bf16,
                           kind="ExternalInput").ap()
    bfc_d = nc.dram_tensor("bfc", [1, D], bf16, kind="ExternalInput").ap()
    hm_d = nc.dram_tensor("hm", [128, 896], bf16, kind="ExternalInput").ap()
    outT = nc.dram_tensor("outT", [D, NHALF * HTOK], f32,
                          kind="ExternalOutput").ap()

    with tile.TileContext(nc) as tc:
        with tc.tile_pool(name="const", bufs=1) as cst, \
             tc.tile_pool(name="dram", bufs=1, space="DRAM") as dpool, \
             tc.tile_pool(name="work", bufs=1) as wk, \
             tc.tile_pool(name="ps", bufs=1, space="PSUM") as ps:

            # ---- weights needed first (QKV of batch 0) ----
            wq = cst.tile([128, 8 * 384], bf16)
            nc.sync.dma_start(wq[:], wq_d[:])
            biasq = cst.tile([1, 3 * 128], bf16)
            nc.sync.dma_start(biasq[:], bq_d[:])
            onesb = cst.tile([1, CH], bf16)
            nc.gpsimd.memset(onesb[:], 1.0)
            # selector for reciprocal broadcast: row 64 = 1, rest 0
            zl = cst.tile([65, 64], bf16)
            nc.gpsimd.memset(zl[:], 0.0)
            nc.gpsimd.memset(zl[64:65, :], 1.0)

            # ---- deferred constants (needed later; don't block first mm) ----
            def _late_consts():
                hm = cst.tile([128, 896], bf16)
                nc.sync.dma_start(hm[:], hm_d[:])
                wfc = cst.tile([128, 64 * 128], bf16)
                nc.sync.dma_start(wfc[:], wfc_d[:])
                biasf = cst.tile([1, D], bf16)
                nc.sync.dma_start(biasf[:], bfc_d[:])
                # per-head reciprocal staging: row 64 = recip, rows 0-63
                # zeroed once (garbage would poison the selector matmul)
                rc = []
                for h in range(HPC):
                    t = cst.tile([65, T], bf16, name=f"rc{h}")
                    nc.gpsimd.memset(t[0:64, :], 0.0)
                    rc.append(t)
                return hm, wfc, biasf, rc

            # ---- per-batch state (double buffered across the pipeline) ----
            def alloc_batch(b):
                qt = wk.tile([128, T], bf16, tag="qt", bufs=2, name=f"qt{b}")
                kt = wk.tile([128, T], bf16, tag="kt", bufs=2, name=f"kt{b}")
                vsb = wk.tile([128, NKV_B * 130], bf16, tag="vsb", bufs=2,
                              name=f"vsb{b}")
                v3 = vsb.rearrange("p (t c) -> p t c", c=130)
                nc.gpsimd.memset(v3[:, :, 64:65], 1.0)
                nc.gpsimd.memset(v3[:, :, 129:130], 1.0)
                praw = [wk.tile([64, T], bf16, tag=f"praw{h}", bufs=2,
                                name=f"praw{h}_{b}") for h in range(HPC)]
                return qt, kt, vsb, praw

            def qkv_dma(b, ch):
                """Prefetch one 512-token x chunk."""
                c0 = b * T + ch * CH
                xt = wk.tile([128, 8 * CH], bf16, tag="xt", bufs=3,
                             name=f"xt{b}_{ch}")
                xt3 = xt.rearrange("p (d c) -> p d c", d=8)
                xs3 = xT[:, c0:c0 + CH].rearrange("(d p) c -> p d c", p=128)
                nc.sync.dma_start(xt3[:], xs3)
                return xt

            def qkv_compute(b, ch, xt, st):
                """Project one 512-token chunk of batch b into qt/kt/vsb."""
                qt, kt, vsb = st[0], st[1], st[2]
                cs = ch * CH
                xt3 = xt.rearrange("p (d c) -> p d c", d=8)
                # Q^T and K^T chunks share one PSUM tile
                psqk = ps.tile([128, 2 * CH], f32, tag="st", bufs=2,
                               name=f"psqk{b}_{ch}")
                for d in range(8):
                    nc.tensor.matmul(psqk[:, 0:CH],
                                     wq[:, d * 384:d * 384 + 128],
                                     xt[:, d * CH:(d + 1) * CH],
                                     start=(d == 0), stop=False)
                nc.tensor.matmul(psqk[:, 0:CH], biasq[0:1, 0:128],
                                 onesb[0:1, :], start=False, stop=True)
                for d in range(8):
                    nc.tensor.matmul(psqk[:, CH:2 * CH],
                                     wq[:, d * 384 + 128:d * 384 + 256],
                                     xt[:, d * CH:(d + 1) * CH],
                                     start=(d == 0), stop=False)
                nc.tensor.matmul(psqk[:, CH:2 * CH], biasq[0:1, 128:256],
                                 onesb[0:1, :], start=False, stop=True)
                nc.vector.tensor_copy(qt[:, cs:cs + CH], psqk[:, 0:CH])
                nc.vector.tensor_copy(kt[:, cs:cs + CH], psqk[:, CH:2 * CH])
                # V token-major: lhsT = x tile, rhs = W_v columns
                psv = ps.tile([128, 2 * CH], f32, tag="st", bufs=2,
                              name=f"psv{b}_{ch}")
                for sb in range(CH // 128):
                    kvt = ch * 4 + sb
                    vo = sb * 128
                    for d in range(8):
                        nc.tensor.matmul(
                            psv[:, vo:vo + 128],
                            xt3[:, d, sb * 128:(sb + 1) * 128],
                            wq[:, d * 384 + 256:d * 384 + 384],
                            start=(d == 0), stop=False)
                    nc.tensor.matmul(psv[:, vo:vo + 128], onesb[0:1, 0:128],
                                     biasq[0:1, 256:384],
                                     start=False, stop=True)
                    base = kvt * 130
                    nc.vector.tensor_copy(vsb[:, base:base + 64],
                                          psv[:, vo:vo + 64])
                    nc.vector.tensor_copy(vsb[:, base + 65:base + 129],
                                          psv[:, vo + 64:vo + 128])

            def attn_qc(b, qc, st, hm, rc):
                """Causal attention for query chunk qc of batch b."""
                qt, kt, vsb, praw = st
                nkv = 4 * (qc + 1)
                pv = [ps.tile([65, CH], f32, tag=f"pv{h}", bufs=1,
                              name=f"pv{h}_{b}_{qc}") for h in range(HPC)]
                for ki in range(nkv):
                    diag = ki - 4 * qc
                    stt = ps.tile([128, 2 * CH], f32, tag="st", bufs=2,
                                  name=f"s_{b}_{qc}_{ki}")
                    pt = wk.tile([128, 2 * CH], bf16, tag="pt", bufs=3,
                                 name=f"p_{b}_{qc}_{ki}")
                    for h in range(HPC):
                        nc.tensor.matmul(
                            stt[:, h * CH:(h + 1) * CH],
                            kt[64 * h:64 * h + 64, ki * 128:(ki + 1) * 128],
                            qt[64 * h:64 * h + 64, qc * CH:(qc + 1) * CH],
                            start=True, stop=True,
                            tile_position=(64 * h, 0))
                    nc.scalar.activation(pt[:], stt[:], EXP, scale=SCALE)
                    if diag >= 0:
                        off = 384 - 128 * diag
                        for h in range(HPC):
                            nc.vector.tensor_mul(
                                pt[:, h * CH:(h + 1) * CH],
                                pt[:, h * CH:(h + 1) * CH],
                                hm[:, off:off + CH])
                    for h in range(HPC):
                        vb = ki * 130 + 65 * h
                        nc.tensor.matmul(pv[h][0:65, :],
                                         vsb[:, vb:vb + 65],
                                         pt[:, h * CH:(h + 1) * CH],
                                         start=(ki == 0),
                                         stop=(ki == nkv - 1))
                # stash raw PV + reciprocal of the denominator row
                for h in range(HPC):
                    nc.vector.tensor_copy(praw[h][:, qc * CH:(qc + 1) * CH],
                                          pv[h][0:64, :])
                    with nc.allow_low_precision(reason="bf16 recip of denom"):
                        nc.vector.reciprocal(
                            rc[h][64:65, qc * CH:(qc + 1) * CH],
                            pv[h][64:65, :])

            def normalize_half(b, hf, st, rc):
                """praw[:, half] *= broadcast(1/denom) via selector matmul."""
                praw = st[3]
                for qc in (2 * hf, 2 * hf + 1):
                    for h in range(HPC):
                        bcq = ps.tile([128, CH], f32, tag="aux", bufs=2,
                                      name=f"bc{h}_{b}_{qc}")
                        nc.tensor.matmul(bcq[0:64, :], zl[:, 0:64],
                                         rc[h][:, qc * CH:(qc + 1) * CH],
                                         start=True, stop=True)
                        rbs = wk.tile([64, CH], bf16, tag="rbs", bufs=2,
                                      name=f"rbs{h}_{b}_{qc}")
                        nc.vector.tensor_copy(rbs[:], bcq[0:64, :])
                        nc.vector.tensor_mul(
                            praw[h][:, qc * CH:(qc + 1) * CH],
                            praw[h][:, qc * CH:(qc + 1) * CH], rbs[:])

            def a2a_half(b, hf, st):
                """Re-shard half-batch (b, hf): head-sharded -> token-sharded."""
                praw = st[3]
                ag_in = dpool.tile([NCORES * 128, HTOK], bf16,
                                   name=f"ag_in{b}_{hf}")
                ag_out = dpool.tile([NCORES * 128, HTOK], bf16,
                                    name=f"ag_out{b}_{hf}")
                div = ag_in.rearrange("(d p) c -> p d c", p=128)
                for h in range(HPC):
                    src = praw[h][:, hf * 1024:(hf + 1) * 1024].rearrange(
                        "p (d c) -> p d c", c=HTOK)
                    nc.sync.dma_start(div[64 * h:64 * h + 64, :, :], src)
                if no_collective:
                    nc.sync.dma_start(ag_out[:], ag_in[:])
                else:
                    nc.gpsimd.collective_compute(
                        "AllToAll", mybir.AluOpType.bypass,
                        replica_groups=[list(range(NCORES))],
                        ins=[ag_in.opt()], outs=[ag_out.opt()])
                return ag_out

            def fc_tokens(b, ag_pair, wfc, biasf):
                """FC for this core's token slice of batch b.

                ag_pair: list of (ag_out, half) — one entry for a half-batch
                FC (moving dim 128, used for the tail), two for a full batch
                (moving dim 256)."""
                ntok = HTOK * len(ag_pair)
                hb0 = 2 * b + ag_pair[0][1]
                tag = f"fci{len(ag_pair)}"
                fci = wk.tile([128, 8 * ntok], bf16, tag=tag, bufs=2,
                              name=f"fci{b}_{hb0}")
                fci3 = fci.rearrange("p (d c) -> p d c", d=8)
                for j, (ag_out, hf) in enumerate(ag_pair):
                    src = ag_out.rearrange("(d p) c -> p d c", p=128)
                    nc.sync.dma_start(
                        fci3[:, :, j * HTOK:(j + 1) * HTOK], src)
                ost = wk.tile([128, 8 * ntok], f32, tag=f"ost{len(ag_pair)}",
                              bufs=2, name=f"ost{b}_{hb0}")
                for fo in range(8):
                    pfc = ps.tile([128, CH], f32, tag="aux", bufs=2,
                                  name=f"pfc{b}_{hb0}_{fo}")
                    for d in range(8):
                        nc.tensor.matmul(
                            pfc[:, 0:ntok],
                            wfc[:, (fo * 8 + d) * 128:(fo * 8 + d + 1) * 128],
                            fci[:, d * ntok:(d + 1) * ntok],
                            start=(d == 0), stop=False)
                    nc.tensor.matmul(pfc[:, 0:ntok],
                                     biasf[0:1, fo * 128:(fo + 1) * 128],
                                     onesb[0:1, 0:ntok],
                                     start=False, stop=True)
                    nc.vector.tensor_copy(ost[:, fo * ntok:(fo + 1) * ntok],
                                          pfc[:, 0:ntok])
                dst = outT.rearrange("(f p) c -> p f c", p=128)[
                    :, :, hb0 * HTOK:hb0 * HTOK + ntok]
                osrc = ost.rearrange("p (f c) -> p f c", c=ntok)
                nc.sync.dma_start(dst, osrc)

            # ================= schedule =================
            states = [None] * B
            states[0] = alloc_batch(0)
            xts = {(0, 0): qkv_dma(0, 0)}
            hm, wfc, biasf, rc = _late_consts()
            for ch in range(NCH_B):
                if ch + 1 < NCH_B:
                    xts[(0, ch + 1)] = qkv_dma(0, ch + 1)
                qkv_compute(0, ch, xts.pop((0, ch)), states[0])

            ag_outs = {}
            for b in range(B):
                if b + 1 < B:
                    states[b + 1] = alloc_batch(b + 1)
                for qc in range(QC):
                    if b + 1 < B:
                        xts[(b + 1, qc)] = qkv_dma(b + 1, qc)
                    attn_qc(b, qc, states[b], hm, rc)
                    if b + 1 < B:
                        qkv_compute(b + 1, qc, xts.pop((b + 1, qc)),
                                    states[b + 1])
                    if qc == 1:
                        normalize_half(b, 0, states[b], rc)
                        ag_outs[(b, 0)] = a2a_half(b, 0, states[b])
                    if qc == 2 and b >= 1:
                        fc_tokens(b - 1, [(ag_outs[(b - 1, 0)], 0),
                                          (ag_outs[(b - 1, 1)], 1)],
                                  wfc, biasf)
                # second half: tail-minimizing split
                if b == B - 1:
                    fc_tokens(b, [(ag_outs[(b, 0)], 0)], wfc, biasf)
                normalize_half(b, 1, states[b], rc)
                ag_outs[(b, 1)] = a2a_half(b, 1, states[b])
            fc_tokens(B - 1, [(ag_outs[(B - 1, 1)], 1)], wfc, biasf)

    nc.compile()
    return nc


def _host_inputs(x, W_qkv, b_qkv, W_fc, b_fc):
    import ml_dtypes
    bf = ml_dtypes.bfloat16
    x = np.asarray(x, dtype=np.float32)
    W_qkv = np.asarray(W_qkv, dtype=np.float32)
    b_qkv = np.asarray(b_qkv, dtype=np.float32)
    W_fc = np.asarray(W_fc, dtype=np.float32)
    b_fc = np.asarray(b_fc, dtype=np.float32)

    xT = np.ascontiguousarray(x.reshape(BT, D).T).astype(bf)
    hm = (np.arange(128)[:, None]
          <= np.arange(896)[None, :] - 384).astype(bf)
    # full FC weights prepacked to SBUF layout [p, (f*8+d)*128 + c]
    wfc = np.ascontiguousarray(
        W_fc.reshape(8, 128, 8, 128).transpose(1, 2, 0, 3).reshape(128, 8192)
    ).astype(bf)
    bfc = np.ascontiguousarray(b_fc[None, :]).astype(bf)
    in_maps = []
    for c in range(NCORES):
        f0 = c * (HPC * HD)  # 128*c
        wqs = np.concatenate(
            [W_qkv[:, p * D + f0: p * D + f0 + 128] for p in range(3)],
            axis=1)  # [1024, 384] = [q|k|v]
        wq_c = np.ascontiguousarray(
            wqs.reshape(8, 128, 384).transpose(1, 0, 2).reshape(128, 3072)
        ).astype(bf)
        bq_c = np.ascontiguousarray(np.concatenate(
            [b_qkv[p * D + f0: p * D + f0 + 128] for p in range(3)])[None, :]
        ).astype(bf)
        in_maps.append({
            "xT": xT, "wq": wq_c, "bq": bq_c, "wfc": wfc, "bfc": bfc,
            "hm": hm,
        })
    return in_maps


def _get_nc():
    if "nc" not in _CACHE:
        _CACHE["nc"] = _build()
    return _CACHE["nc"]


def _assemble(results):
    full = np.empty((BT, D), dtype=np.float32)
    for c in range(NCORES):
        o = results[c]["outT"]  # [1024 features, 8 half-batches * 128 tokens]
        for b in range(B):
            for hf in range(2):
                t0 = b * T + hf * 1024 + c * HTOK
                full[t0:t0 + HTOK, :] = \
                    o[:, (2 * b + hf) * HTOK:(2 * b + hf + 1) * HTOK].T
    return full.reshape(B, T, D)


def kernel(x, W_qkv, b_qkv, W_fc, b_fc):
    nc = _get_nc()
    in_maps = _host_inputs(x, W_qkv, b_qkv, W_fc, b_fc)
    res = run_bass_kernel_spmd(nc, in_maps, list(range(NCORES)))
    return _assemble(res.results)


# revision 11
# speedup vs baseline: 1.5431x; 1.0471x over previous
"""Trainium2 Bass kernel for causal multi-head attention (B=4, T=2048, D=1024, H=16).

Sharding: tensor-parallel over heads for QKV+attention (each of 8 cores owns
2 heads over all tokens), then AllToAll re-shards from head-sharded to
token-sharded so each core computes the final FC over the full feature dim
for its 256-token slice of each batch.

All matmuls run in bf16 (fp32 streams at half PE rate; bf16 at full), with
fp32 PSUM accumulation. Scores are computed transposed (S^T = K Q^T, two
heads packed in PE quadrants via tile_position) so softmax normalization
lands on the PV matmul's free dim; the denominator comes from a ones column
augmented into V. Normalization is deferred out of the inner loop (the
per-chunk broadcast chain caused PE bubbles + HAM re-throttling). The
denominator reciprocal runs on the scalar engine (DVE reciprocal is 3.3us
for [1,512] and stalls the next chunk via a PSUM WAR; custom-DVE
reciprocal_approx_fast and AluOp divide are broken/unsupported on this
stack), and the broadcast across partitions uses a selector matmul
(partition-shifted DVE/DMA ops misbehave; SBUF-side DMA access patterns
must keep the partition dim outermost). Mask-multiplies and normalize
multiplies run on the otherwise-idle GPSIMD engine (it cannot read PSUM).
One AllToAll per batch (smaller per-half collectives measured ~3x slower
per byte), and QKV of batch b+1 / FC of batch b-1 interleave into batch
b's attention to keep the PE fed while the scalar engine runs exp.
"""
import sys

for _p in ("/opt/trn_rl_repo",):
    if _p not in sys.path:
        sys.path.insert(0, _p)

import numpy as np

import concourse.bass as bass
import concourse.mybir as mybir
import concourse.tile as tile
from concourse import bacc
from concourse.bass_utils import run_bass_kernel_spmd

f32 = mybir.dt.float32
bf16 = mybir.dt.bfloat16
EXP = mybir.ActivationFunctionType.Exp

B, T, D, H, HD = 4, 2048, 1024, 16, 64
NCORES = 8
HPC = H // NCORES          # heads per core = 2
BT = B * T                 # 8192
CH = 512                   # token chunk (q chunk / projection chunk)
NCH_B = T // CH            # 4 projection chunks per batch
QC = T // CH               # 4 query chunks per batch
NKV_B = T // 128           # 16 kv tiles of 128 per batch
TOK = 256                  # tokens per core per batch (after AllToAll)
SCALE = 1.0 / 8.0          # 1/sqrt(HD)

RECIP_MODE = "scalar"

_CACHE = {}


def _build(no_collective=False):
    nc = bacc.Bacc("TRN2", target_bir_lowering=False, debug=False,
                   num_devices=NCORES)

    xT = nc.dram_tensor("xT", [D, BT], bf16, kind="ExternalInput").ap()
    wq_d = nc.dram_tensor("wq", [128, 8 * 384], bf16, kind="ExternalInput").ap()
    bq_d = nc.dram_tensor("bq", [1, 3 * 128], bf16, kind="ExternalInput").ap()
    wfc_d = nc.dram_tensor("wfc", [128, 64 * 128], bf16,
                           kind="ExternalInput").ap()
    bfc_d = nc.dram_tensor("bfc", [1, D], bf16, kind="ExternalInput").ap()
    hm_d = nc.dram_tensor("hm", [128, 896], bf16, kind="ExternalInput").ap()
    outT = nc.dram_tensor("outT", [D, B * TOK], f32,
                          kind="ExternalOutput").ap()

    with tile.TileContext(nc) as tc:
        with tc.tile_pool(name="const", bufs=1) as cst, \
             tc.tile_pool(name="dram", bufs=1, space="DRAM") as dpool, \
             tc.tile_pool(name="work", bufs=1) as wk, \
             tc.tile_pool(name="ps", bufs=1, space="PSUM") as ps:

            # ---- weights needed first (QKV of batch 0) ----
            wq = cst.tile([128, 8 * 384], bf16)
            nc.sync.dma_start(wq[:], wq_d[:])
            biasq = cst.tile([1, 3 * 128], bf16)
            nc.sync.dma_start(biasq[:], bq_d[:])
            onesb = cst.tile([1, CH], bf16)
            nc.gpsimd.memset(onesb[:], 1.0)
            # selector for reciprocal broadcast: row 64 = 1, rest 0
            zl = cst.tile([65, 64], bf16)
            nc.gpsimd.memset(zl[:], 0.0)
            nc.gpsimd.memset(zl[64:65, :], 1.0)

            # ---- deferred constants (needed later; don't block first mm) ----
            def _late_consts():
                hm = cst.tile([128, 896], bf16)
                nc.sync.dma_start(hm[:], hm_d[:])
                wfc = cst.tile([128, 64 * 128], bf16)
                nc.sync.dma_start(wfc[:], wfc_d[:])
                biasf = cst.tile([1, D], bf16)
                nc.sync.dma_start(biasf[:], bfc_d[:])
                # per-head reciprocal staging: row 64 = recip, rows 0-63
                # zeroed once (garbage would poison the selector matmul)
                rc = []
                for h in range(HPC):
                    t = cst.tile([65, T], bf16, name=f"rc{h}")
                    nc.gpsimd.memset(t[0:64, :], 0.0)
                    rc.append(t)
                return hm, wfc, biasf, rc

            # ---- per-batch state (double buffered across the pipeline) ----
            def alloc_batch(b):
                qt = wk.tile([128, T], bf16, tag="qt", bufs=2, name=f"qt{b}")
                kt = wk.tile([128, T], bf16, tag="kt", bufs=2, name=f"kt{b}")
                vsb = wk.tile([128, NKV_B * 130], bf16, tag="vsb", bufs=2,
                              name=f"vsb{b}")
                v3 = vsb.rearrange("p (t c) -> p t c", c=130)
                nc.gpsimd.memset(v3[:, :, 64:65], 1.0)
                nc.gpsimd.memset(v3[:, :, 129:130], 1.0)
                praw = [wk.tile([64, T], bf16, tag=f"praw{h}", bufs=2,
                                name=f"praw{h}_{b}") for h in range(HPC)]
                return qt, kt, vsb, praw

            def qkv_dma(b, ch):
                """Prefetch one 512-token x chunk."""
                c0 = b * T + ch * CH
                xt = wk.tile([128, 8 * CH], bf16, tag="xt", bufs=3,
                             name=f"xt{b}_{ch}")
                xt3 = xt.rearrange("p (d c) -> p d c", d=8)
                xs3 = xT[:, c0:c0 + CH].rearrange("(d p) c -> p d c", p=128)
                nc.sync.dma_start(xt3[:], xs3)
                return xt

            def qkv_compute(b, ch, xt, st):
                """Project one 512-token chunk of batch b into qt/kt/vsb."""
                qt, kt, vsb = st[0], st[1], st[2]
                cs = ch * CH
                xt3 = xt.rearrange("p (d c) -> p d c", d=8)
                # Q^T and K^T chunks share one PSUM tile
                psqk = ps.tile([128, 2 * CH], f32, tag="st", bufs=2,
                               name=f"psqk{b}_{ch}")
                for d in range(8):
                    nc.tensor.matmul(psqk[:, 0:CH],
                                     wq[:, d * 384:d * 384 + 128],
                                     xt[:, d * CH:(d + 1) * CH],
                                     start=(d == 0), stop=False)
                nc.tensor.matmul(psqk[:, 0:CH], biasq[0:1, 0:128],
                                 onesb[0:1, :], start=False, stop=True)
                for d in range(8):
                    nc.tensor.matmul(psqk[:, CH:2 * CH],
                                     wq[:, d * 384 + 128:d * 384 + 256],
                                     xt[:, d * CH:(d + 1) * CH],
                                     start=(d == 0), stop=False)
                nc.tensor.matmul(psqk[:, CH:2 * CH], biasq[0:1, 128:256],
                                 onesb[0:1, :], start=False, stop=True)
                nc.vector.tensor_copy(qt[:, cs:cs + CH], psqk[:, 0:CH])
                nc.vector.tensor_copy(kt[:, cs:cs + CH], psqk[:, CH:2 * CH])
                # V token-major: lhsT = x tile, rhs = W_v columns
                psv = ps.tile([128, 2 * CH], f32, tag="st", bufs=2,
                              name=f"psv{b}_{ch}")
                for sb in range(CH // 128):
                    kvt = ch * 4 + sb
                    vo = sb * 128
                    for d in range(8):
                        nc.tensor.matmul(
                            psv[:, vo:vo + 128],
                            xt3[:, d, sb * 128:(sb + 1) * 128],
                            wq[:, d * 384 + 256:d * 384 + 384],
                            start=(d == 0), stop=False)
                    nc.tensor.matmul(psv[:, vo:vo + 128], onesb[0:1, 0:128],
                                     biasq[0:1, 256:384],
                                     start=False, stop=True)
                    base = kvt * 130
                    nc.vector.tensor_copy(vsb[:, base:base + 64],
                                          psv[:, vo:vo + 64])
                    nc.vector.tensor_copy(vsb[:, base + 65:base + 129],
                                          psv[:, vo + 64:vo + 128])

            def _recip(out, in_):
                if RECIP_MODE == "scalar":
                    # ScalarE table-based reciprocal: ~0.7us for [1,512] vs
                    # 3.3us on DVE; accuracy (~1e-3) is plenty for the bf16
                    # broadcast that consumes it.
                    imm = mybir.ImmediateValue
                    nc.scalar.add_instruction(
                        mybir.InstActivation(
                            name=nc.get_next_instruction_name(),
                            func=mybir.ActivationFunctionType.Reciprocal,
                            ins=[nc.scalar.lower_ap(in_),
                                 imm(dtype=mybir.dt.float32, value=0.0),
                                 imm(dtype=mybir.dt.float32, value=1.0),
                                 imm(dtype=mybir.dt.float32, value=0.0)],
                            outs=[nc.scalar.lower_ap(out)],
                        ))
                else:
                    with nc.allow_low_precision(reason="bf16 denom recip"):
                        nc.vector.reciprocal(out, in_)

            def attn_qc(b, qc, st, hm, rc):
                """Causal attention for query chunk qc of batch b."""
                qt, kt, vsb, praw = st
                nkv = 4 * (qc + 1)
                pv = [ps.tile([65, CH], f32, tag=f"pv{h}", bufs=1,
                              name=f"pv{h}_{b}_{qc}") for h in range(HPC)]
                for ki in range(nkv):
                    diag = ki - 4 * qc
                    # on diagonal tiles, queries below the block are fully
                    # masked: shrink the moving dim to the causal range
                    off = 128 * diag if diag > 0 else 0
                    m = CH - off
                    stt = ps.tile([128, 2 * CH], f32, tag="st", bufs=2,
                                  name=f"s_{b}_{qc}_{ki}")
                    pt = wk.tile([128, 2 * CH], bf16, tag="pt", bufs=3,
                                 name=f"p_{b}_{qc}_{ki}")
                    for h in range(HPC):
                        nc.tensor.matmul(
                            stt[:, h * CH + off:(h + 1) * CH],
                            kt[64 * h:64 * h + 64, ki * 128:(ki + 1) * 128],
                            qt[64 * h:64 * h + 64,
                               qc * CH + off:(qc + 1) * CH],
                            start=True, stop=True,
                            tile_position=(64 * h, 0))
                    if diag > 0:
                        for h in range(HPC):
                            nc.scalar.activation(
                                pt[:, h * CH + off:(h + 1) * CH],
                                stt[:, h * CH + off:(h + 1) * CH],
                                EXP, scale=SCALE)
                    else:
                        nc.scalar.activation(pt[:], stt[:], EXP, scale=SCALE)
                    if diag >= 0:
                        for h in range(HPC):
                            nc.gpsimd.tensor_mul(
                                pt[:, h * CH + off:(h + 1) * CH],
                                pt[:, h * CH + off:(h + 1) * CH],
                                hm[:, 384:384 + m])
                    for h in range(HPC):
                        vb = ki * 130 + 65 * h
                        nc.tensor.matmul(pv[h][0:65, off:CH],
                                         vsb[:, vb:vb + 65],
                                         pt[:, h * CH + off:(h + 1) * CH],
                                         start=(ki == 0),
                                         stop=(ki == nkv - 1))
                # stash raw PV + reciprocal of the denominator row
                for h in range(HPC):
                    nc.vector.tensor_copy(praw[h][:, qc * CH:(qc + 1) * CH],
                                          pv[h][0:64, :])
                    _recip(rc[h][64:65, qc * CH:(qc + 1) * CH],
                           pv[h][64:65, :])

            def normalize_batch(b, st, rc):
                """praw *= broadcast(1/denom) via selector matmul."""
                praw = st[3]
                for qc in range(QC):
                    for h in range(HPC):
                        bcq = ps.tile([128, CH], f32, tag="aux", bufs=2,
                                      name=f"bc{h}_{b}_{qc}")
                        nc.tensor.matmul(bcq[0:64, :], zl[:, 0:64],
                                         rc[h][:, qc * CH:(qc + 1) * CH],
                                         start=True, stop=True)
                        rbs = wk.tile([64, CH], bf16, tag="rbs", bufs=2,
                                      name=f"rbs{h}_{b}_{qc}")
                        nc.vector.tensor_copy(rbs[:], bcq[0:64, :])
                        nc.gpsimd.tensor_mul(
                            praw[h][:, qc * CH:(qc + 1) * CH],
                            praw[h][:, qc * CH:(qc + 1) * CH], rbs[:])

            def a2a_batch(b, st):
                """Re-shard batch b attention output: head- to token-sharded."""
                praw = st[3]
                ag_in = dpool.tile([NCORES * 128, TOK], bf16,
                                   name=f"ag_in{b}")
                ag_out = dpool.tile([NCORES * 128, TOK], bf16,
                                    name=f"ag_out{b}")
                div = ag_in.rearrange("(d p) c -> p d c", p=128)
                for h in range(HPC):
                    src = praw[h].rearrange("p (d c) -> p d c", c=TOK)
                    nc.sync.dma_start(div[64 * h:64 * h + 64, :, :], src)
                if no_collective:
                    nc.sync.dma_start(ag_out[:], ag_in[:])
                else:
                    nc.gpsimd.collective_compute(
                        "AllToAll", mybir.AluOpType.bypass,
                        replica_groups=[list(range(NCORES))],
                        ins=[ag_in.opt()], outs=[ag_out.opt()])
                return ag_out

            def fc_batch(b, ag_out, wfc, biasf):
                """Full FC for this core's 256-token slice of batch b."""
                fci = wk.tile([128, 8 * TOK], bf16, tag="fci", bufs=2,
                              name=f"fci{b}")
                fci3 = fci.rearrange("p (d c) -> p d c", d=8)
                src = ag_out.rearrange("(d p) c -> p d c", p=128)
                nc.sync.dma_start(fci3[:], src)
                ost = wk.tile([128, 8 * TOK], f32, tag="ost", bufs=2,
                              name=f"ost{b}")
                for fo in range(8):
                    pfc = ps.tile([128, CH], f32, tag="aux", bufs=2,
                                  name=f"pfc{b}_{fo}")
                    for d in range(8):
                        nc.tensor.matmul(
                            pfc[:, 0:TOK],
                            wfc[:, (fo * 8 + d) * 128:(fo * 8 + d + 1) * 128],
                            fci[:, d * TOK:(d + 1) * TOK],
                            start=(d == 0), stop=False)
                    nc.tensor.matmul(pfc[:, 0:TOK],
                                     biasf[0:1, fo * 128:(fo + 1) * 128],
                                     onesb[0:1, 0:TOK],
                                     start=False, stop=True)
                    nc.vector.tensor_copy(ost[:, fo * TOK:(fo + 1) * TOK],
                                          pfc[:, 0:TOK])
                dst = outT.rearrange("(f p) c -> p f c", p=128)[
                    :, :, b * TOK:(b + 1) * TOK]
                osrc = ost.rearrange("p (f c) -> p f c", c=TOK)
                nc.sync.dma_start(dst, osrc)

            # ================= schedule =================
            states = [None] * B
            states[0] = alloc_batch(0)
            xts = {(0, 0): qkv_dma(0, 0)}
            hm, wfc, biasf, rc = _late_consts()
            for ch in range(NCH_B):
                if ch + 1 < NCH_B:
                    xts[(0, ch + 1)] = qkv_dma(0, ch + 1)
                qkv_compute(0, ch, xts.pop((0, ch)), states[0])

            ag_outs = [None] * B
            for b in range(B):
                if b + 1 < B:
                    states[b + 1] = alloc_batch(b + 1)
                for qc in range(QC):
                    if b + 1 < B:
                        xts[(b + 1, qc)] = qkv_dma(b + 1, qc)
                    attn_qc(b, qc, states[b], hm, rc)
                    if b + 1 < B:
                        qkv_compute(b + 1, qc, xts.pop((b + 1, qc)),
                                    states[b + 1])
                    if qc == 2 and b >= 1:
                        fc_batch(b - 1, ag_outs[b - 1], wfc, biasf)
                normalize_batch(b, states[b], rc)
                ag_outs[b] = a2a_batch(b, states[b])
            fc_batch(B - 1, ag_outs[B - 1], wfc, biasf)

    nc.compile()
    return nc


def _host_inputs(x, W_qkv, b_qkv, W_fc, b_fc):
    import ml_dtypes
    bf = ml_dtypes.bfloat16
    x = np.asarray(x, dtype=np.float32)
    W_qkv = np.asarray(W_qkv, dtype=np.float32)
    b_qkv = np.asarray(b_qkv, dtype=np.float32)
    W_fc = np.asarray(W_fc, dtype=np.float32)
    b_fc = np.asarray(b_fc, dtype=np.float32)

    xT = np.ascontiguousarray(x.reshape(BT, D).T).astype(bf)
    hm = (np.arange(128)[:, None]
          <= np.arange(896)[None, :] - 384).astype(bf)
    # full FC weights prepacked to SBUF layout [p, (f*8+d)*128 + c]
    wfc = np.ascontiguousarray(
        W_fc.reshape(8, 128, 8, 128).transpose(1, 2, 0, 3).reshape(128, 8192)
    ).astype(bf)
    bfc = np.ascontiguousarray(b_fc[None, :]).astype(bf)
    in_maps = []
    for c in range(NCORES):
        f0 = c * (HPC * HD)  # 128*c
        wqs = np.concatenate(
            [W_qkv[:, p * D + f0: p * D + f0 + 128] for p in range(3)],
            axis=1)  # [1024, 384] = [q|k|v]
        wq_c = np.ascontiguousarray(
            wqs.reshape(8, 128, 384).transpose(1, 0, 2).reshape(128, 3072)
        ).astype(bf)
        bq_c = np.ascontiguousarray(np.concatenate(
            [b_qkv[p * D + f0: p * D + f0 + 128] for p in range(3)])[None, :]
        ).astype(bf)
        in_maps.append({
            "xT": xT, "wq": wq_c, "bq": bq_c, "wfc": wfc, "bfc": bfc,
            "hm": hm,
        })
    return in_maps


def _get_nc():
    if "nc" not in _CACHE:
        _CACHE["nc"] = _build()
    return _CACHE["nc"]


def _assemble(results):
    full = np.empty((BT, D), dtype=np.float32)
    for c in range(NCORES):
        o = results[c]["outT"]  # [1024 features, 4*256 tokens]
        for b in range(B):
            full[b * T + c * TOK: b * T + (c + 1) * TOK, :] = \
                o[:, b * TOK:(b + 1) * TOK].T
    return full.reshape(B, T, D)


def kernel(x, W_qkv, b_qkv, W_fc, b_fc):
    nc = _get_nc()
    in_maps = _host_inputs(x, W_qkv, b_qkv, W_fc, b_fc)
    res = run_bass_kernel_spmd(nc, in_maps, list(range(NCORES)))
    return _assemble(res.results)


# revision 13
# speedup vs baseline: 1.6255x; 1.0534x over previous
"""Trainium2 Bass kernel for causal multi-head attention (B=4, T=2048, D=1024, H=16).

Sharding: tensor-parallel over heads for QKV+attention (each of 8 cores owns
2 heads over all tokens), then AllToAll re-shards from head-sharded to
token-sharded so each core computes the final FC over the full feature dim
for its 256-token slice of each batch.

All matmuls run in bf16 (fp32 streams at half PE rate; bf16 at full), with
fp32 PSUM accumulation. Scores are computed transposed (S^T = K Q^T, two
heads packed in PE quadrants via tile_position) so softmax normalization
lands on the PV matmul's free dim; the denominator comes from a ones column
augmented into V. Normalization is deferred out of the inner loop (the
per-chunk broadcast chain caused PE bubbles + HAM re-throttling). The
denominator reciprocal runs on the scalar engine (DVE reciprocal is 3.3us
for [1,512] and stalls the next chunk via a PSUM WAR; custom-DVE
reciprocal_approx_fast and AluOp divide are broken/unsupported on this
stack), and the broadcast across partitions uses a selector matmul
(partition-shifted DVE/DMA ops misbehave; SBUF-side DMA access patterns
must keep the partition dim outermost). Mask-multiplies and normalize
multiplies run on the otherwise-idle GPSIMD engine (it cannot read PSUM).
One AllToAll per batch (smaller per-half collectives measured ~3x slower
per byte), and QKV of batch b+1 / FC of batch b-1 interleave into batch
b's attention to keep the PE fed while the scalar engine runs exp.
"""
import sys

for _p in ("/opt/trn_rl_repo",):
    if _p not in sys.path:
        sys.path.insert(0, _p)

import numpy as np

import concourse.bass as bass
import concourse.mybir as mybir
import concourse.tile as tile
from concourse import bacc
from concourse.bass_utils import run_bass_kernel_spmd

f32 = mybir.dt.float32
bf16 = mybir.dt.bfloat16
EXP = mybir.ActivationFunctionType.Exp

B, T, D, H, HD = 4, 2048, 1024, 16, 64
NCORES = 8
HPC = H // NCORES          # heads per core = 2
BT = B * T                 # 8192
CH = 512                   # token chunk (q chunk / projection chunk)
NCH_B = T // CH            # 4 projection chunks per batch
QC = T // CH               # 4 query chunks per batch
NKV_B = T // 128           # 16 kv tiles of 128 per batch
TOK = 256                  # tokens per core per batch (after AllToAll)
SCALE = 1.0 / 8.0          # 1/sqrt(HD)

RECIP_MODE = "scalar"

_CACHE = {}


def _build(no_collective=False):
    nc = bacc.Bacc("TRN2", target_bir_lowering=False, debug=False,
                   num_devices=NCORES)

    xT = nc.dram_tensor("xT", [D, BT], bf16, kind="ExternalInput").ap()
    wq_d = nc.dram_tensor("wq", [128, 8 * 384], bf16, kind="ExternalInput").ap()
    bq_d = nc.dram_tensor("bq", [1, 3 * 128], bf16, kind="ExternalInput").ap()
    wfc_d = nc.dram_tensor("wfc", [128, 64 * 128], bf16,
                           kind="ExternalInput").ap()
    bfc_d = nc.dram_tensor("bfc", [1, D], bf16, kind="ExternalInput").ap()
    hm_d = nc.dram_tensor("hm", [128, 896], bf16, kind="ExternalInput").ap()
    outT = nc.dram_tensor("outT", [D, B * TOK], f32,
                          kind="ExternalOutput").ap()

    with tile.TileContext(nc) as tc:
        with tc.tile_pool(name="const", bufs=1) as cst, \
             tc.tile_pool(name="dram", bufs=1, space="DRAM") as dpool, \
             tc.tile_pool(name="work", bufs=1) as wk, \
             tc.tile_pool(name="ps", bufs=1, space="PSUM") as ps:

            # ---- weights needed first (QKV of batch 0) ----
            wq = cst.tile([128, 8 * 384], bf16)
            nc.sync.dma_start(wq[:], wq_d[:])
            biasq = cst.tile([1, 3 * 128], bf16)
            nc.sync.dma_start(biasq[:], bq_d[:])
            onesb = cst.tile([1, CH], bf16)
            nc.gpsimd.memset(onesb[:], 1.0)
            # selector for reciprocal broadcast: row 64 = 1, rest 0
            zl = cst.tile([65, 64], bf16)
            nc.gpsimd.memset(zl[:], 0.0)
            nc.gpsimd.memset(zl[64:65, :], 1.0)

            # ---- deferred constants (needed later; don't block first mm) ----
            def _late_consts():
                hm = cst.tile([128, 896], bf16)
                nc.sync.dma_start(hm[:], hm_d[:])
                wfc = cst.tile([128, 64 * 128], bf16)
                nc.sync.dma_start(wfc[:], wfc_d[:])
                biasf = cst.tile([1, D], bf16)
                nc.sync.dma_start(biasf[:], bfc_d[:])
                # per-head reciprocal staging: row 64 = recip, rows 0-63
                # zeroed once (garbage would poison the selector matmul)
                rc = []
                for h in range(HPC):
                    t = cst.tile([65, T], bf16, name=f"rc{h}")
                    nc.gpsimd.memset(t[0:64, :], 0.0)
                    rc.append(t)
                return hm, wfc, biasf, rc

            # ---- per-batch state (double buffered across the pipeline) ----
            def alloc_batch(b):
                qt = wk.tile([128, T], bf16, tag="qt", bufs=2, name=f"qt{b}")
                kt = wk.tile([128, T], bf16, tag="kt", bufs=2, name=f"kt{b}")
                vsb = wk.tile([128, NKV_B * 130], bf16, tag="vsb", bufs=2,
                              name=f"vsb{b}")
                v3 = vsb.rearrange("p (t c) -> p t c", c=130)
                nc.gpsimd.memset(v3[:, :, 64:65], 1.0)
                nc.gpsimd.memset(v3[:, :, 129:130], 1.0)
                praw = [wk.tile([64, T], bf16, tag=f"praw{h}", bufs=2,
                                name=f"praw{h}_{b}") for h in range(HPC)]
                return qt, kt, vsb, praw

            def qkv_dma(b, ch):
                """Prefetch one 512-token x chunk."""
                c0 = b * T + ch * CH
                xt = wk.tile([128, 8 * CH], bf16, tag="xt", bufs=3,
                             name=f"xt{b}_{ch}")
                xt3 = xt.rearrange("p (d c) -> p d c", d=8)
                xs3 = xT[:, c0:c0 + CH].rearrange("(d p) c -> p d c", p=128)
                nc.sync.dma_start(xt3[:], xs3)
                return xt

            def qkv_units(b, ch, xt, st):
                """Chunk projection as self-contained PE units (aux PSUM).

                Each unit allocates its own short-lived [128,512] aux tile so
                units can interleave with attention without pinning the
                st-tag rotation."""
                qt, kt, vsb = st[0], st[1], st[2]
                cs = ch * CH
                xt3 = xt.rearrange("p (d c) -> p d c", d=8)

                def q_unit(tgt, wo, bo):
                    def emit():
                        pq = ps.tile([128, CH], f32, tag="aux", bufs=2,
                                     name=f"pq{b}_{ch}_{wo}")
                        for d in range(8):
                            nc.tensor.matmul(pq[:],
                                             wq[:, d * 384 + wo:d * 384 + wo + 128],
                                             xt[:, d * CH:(d + 1) * CH],
                                             start=(d == 0), stop=False)
                        nc.tensor.matmul(pq[:], biasq[0:1, bo:bo + 128],
                                         onesb[0:1, :], start=False, stop=True)
                        nc.vector.tensor_copy(tgt[:, cs:cs + CH], pq[:])
                    return emit

                def v_unit(sb):
                    def emit():
                        kvt = ch * 4 + sb
                        pvv = ps.tile([128, CH], f32, tag="aux", bufs=2,
                                      name=f"pvv{b}_{ch}_{sb}")
                        for d in range(8):
                            nc.tensor.matmul(
                                pvv[:, 0:128],
                                xt3[:, d, sb * 128:(sb + 1) * 128],
                                wq[:, d * 384 + 256:d * 384 + 384],
                                start=(d == 0), stop=False)
                        nc.tensor.matmul(pvv[:, 0:128], onesb[0:1, 0:128],
                                         biasq[0:1, 256:384],
                                         start=False, stop=True)
                        base = kvt * 130
                        nc.vector.tensor_copy(vsb[:, base:base + 64],
                                              pvv[:, 0:64])
                        nc.vector.tensor_copy(vsb[:, base + 65:base + 129],
                                              pvv[:, 64:128])
                    return emit

                return [q_unit(qt, 0, 0), q_unit(kt, 128, 128),
                        v_unit(0), v_unit(1), v_unit(2), v_unit(3)]

            def _recip(out, in_):
                if RECIP_MODE == "scalar":
                    # ScalarE table-based reciprocal: ~0.7us for [1,512] vs
                    # 3.3us on DVE; accuracy (~1e-3) is plenty for the bf16
                    # broadcast that consumes it.
                    imm = mybir.ImmediateValue
                    nc.scalar.add_instruction(
                        mybir.InstActivation(
                            name=nc.get_next_instruction_name(),
                            func=mybir.ActivationFunctionType.Reciprocal,
                            ins=[nc.scalar.lower_ap(in_),
                                 imm(dtype=mybir.dt.float32, value=0.0),
                                 imm(dtype=mybir.dt.float32, value=1.0),
                                 imm(dtype=mybir.dt.float32, value=0.0)],
                            outs=[nc.scalar.lower_ap(out)],
                        ))
                else:
                    with nc.allow_low_precision(reason="bf16 denom recip"):
                        nc.vector.reciprocal(out, in_)

            def attn_qc(b, qc, st, hm, rc, drain):
                """Causal attention for query chunk qc of batch b."""
                qt, kt, vsb, praw = st
                nkv = 4 * (qc + 1)
                pv = [ps.tile([65, CH], f32, tag=f"pv{h}", bufs=1,
                              name=f"pv{h}_{b}_{qc}") for h in range(HPC)]
                for ki in range(nkv):
                    diag = ki - 4 * qc
                    # on diagonal tiles, queries below the block are fully
                    # masked: shrink the moving dim to the causal range
                    off = 128 * diag if diag > 0 else 0
                    m = CH - off
                    stt = ps.tile([128, 2 * CH], f32, tag="st", bufs=2,
                                  name=f"s_{b}_{qc}_{ki}")
                    pt = wk.tile([128, 2 * CH], bf16, tag="pt", bufs=3,
                                 name=f"p_{b}_{qc}_{ki}")
                    for h in range(HPC):
                        nc.tensor.matmul(
                            stt[:, h * CH + off:(h + 1) * CH],
                            kt[64 * h:64 * h + 64, ki * 128:(ki + 1) * 128],
                            qt[64 * h:64 * h + 64,
                               qc * CH + off:(qc + 1) * CH],
                            start=True, stop=True,
                            tile_position=(64 * h, 0))
                    if diag > 0:
                        for h in range(HPC):
                            nc.scalar.activation(
                                pt[:, h * CH + off:(h + 1) * CH],
                                stt[:, h * CH + off:(h + 1) * CH],
                                EXP, scale=SCALE)
                    else:
                        nc.scalar.activation(pt[:], stt[:], EXP, scale=SCALE)
                    if diag >= 0:
                        for h in range(HPC):
                            nc.gpsimd.tensor_mul(
                                pt[:, h * CH + off:(h + 1) * CH],
                                pt[:, h * CH + off:(h + 1) * CH],
                                hm[:, 384:384 + m])
                    for h in range(HPC):
                        vb = ki * 130 + 65 * h
                        nc.tensor.matmul(pv[h][0:65, off:CH],
                                         vsb[:, vb:vb + 65],
                                         pt[:, h * CH + off:(h + 1) * CH],
                                         start=(ki == 0),
                                         stop=(ki == nkv - 1))
                    drain(1)
                # stash raw PV + reciprocal of the denominator row
                for h in range(HPC):
                    nc.vector.tensor_copy(praw[h][:, qc * CH:(qc + 1) * CH],
                                          pv[h][0:64, :])
                    _recip(rc[h][64:65, qc * CH:(qc + 1) * CH],
                           pv[h][64:65, :])

            def normalize_batch(b, st, rc):
                """praw *= broadcast(1/denom) via selector matmul."""
                praw = st[3]
                for qc in range(QC):
                    for h in range(HPC):
                        bcq = ps.tile([128, CH], f32, tag="aux", bufs=2,
                                      name=f"bc{h}_{b}_{qc}")
                        nc.tensor.matmul(bcq[0:64, :], zl[:, 0:64],
                                         rc[h][:, qc * CH:(qc + 1) * CH],
                                         start=True, stop=True)
                        rbs = wk.tile([64, CH], bf16, tag="rbs", bufs=2,
                                      name=f"rbs{h}_{b}_{qc}")
                        nc.vector.tensor_copy(rbs[:], bcq[0:64, :])
                        nc.gpsimd.tensor_mul(
                            praw[h][:, qc * CH:(qc + 1) * CH],
                            praw[h][:, qc * CH:(qc + 1) * CH], rbs[:])

            def a2a_batch(b, st):
                """Re-shard batch b attention output: head- to token-sharded."""
                praw = st[3]
                ag_in = dpool.tile([NCORES * 128, TOK], bf16,
                                   name=f"ag_in{b}")
                ag_out = dpool.tile([NCORES * 128, TOK], bf16,
                                    name=f"ag_out{b}")
                div = ag_in.rearrange("(d p) c -> p d c", p=128)
                for h in range(HPC):
                    src = praw[h].rearrange("p (d c) -> p d c", c=TOK)
                    nc.sync.dma_start(div[64 * h:64 * h + 64, :, :], src)
                if no_collective:
                    nc.sync.dma_start(ag_out[:], ag_in[:])
                else:
                    nc.gpsimd.collective_compute(
                        "AllToAll", mybir.AluOpType.bypass,
                        replica_groups=[list(range(NCORES))],
                        ins=[ag_in.opt()], outs=[ag_out.opt()])
                return ag_out

            def fc_units(b, ag_out, wfc, biasf):
                """Full FC for this core's 256-token slice, as PE units."""
                box = {}

                def load():
                    fci = wk.tile([128, 8 * TOK], bf16, tag="fci", bufs=2,
                                  name=f"fci{b}")
                    fci3 = fci.rearrange("p (d c) -> p d c", d=8)
                    srcv = ag_out.rearrange("(d p) c -> p d c", p=128)
                    nc.sync.dma_start(fci3[:], srcv)
                    box["fci"] = fci
                    box["ost"] = wk.tile([128, 8 * TOK], f32, tag="ost",
                                         bufs=2, name=f"ost{b}")

                def fo_unit(fo):
                    def emit():
                        fci, ost = box["fci"], box["ost"]
                        pfc = ps.tile([128, CH], f32, tag="aux", bufs=2,
                                      name=f"pfc{b}_{fo}")
                        for d in range(8):
                            nc.tensor.matmul(
                                pfc[:, 0:TOK],
                                wfc[:, (fo * 8 + d) * 128:
                                     (fo * 8 + d + 1) * 128],
                                fci[:, d * TOK:(d + 1) * TOK],
                                start=(d == 0), stop=False)
                        nc.tensor.matmul(pfc[:, 0:TOK],
                                         biasf[0:1, fo * 128:(fo + 1) * 128],
                                         onesb[0:1, 0:TOK],
                                         start=False, stop=True)
                        nc.vector.tensor_copy(
                            ost[:, fo * TOK:(fo + 1) * TOK], pfc[:, 0:TOK])
                    return emit

                def store():
                    dst = outT.rearrange("(f p) c -> p f c", p=128)[
                        :, :, b * TOK:(b + 1) * TOK]
                    osrc = box["ost"].rearrange("p (f c) -> p f c", c=TOK)
                    nc.sync.dma_start(dst, osrc)

                return [load] + [fo_unit(fo) for fo in range(8)] + [store]

            def dummy_unit(tag_i):
                """~0.9us of dependency-free matmuls to keep the HAM warm."""
                def emit():
                    pdm = ps.tile([128, CH], f32, tag="aux", bufs=2,
                                  name=f"pdm{tag_i}")
                    for r in range(4):
                        nc.tensor.matmul(pdm[0:1, :], onesb[0:1, 0:1],
                                         onesb[0:1, :],
                                         start=True, stop=True)
                return emit

            # ================= schedule =================
            from collections import deque
            filler = deque()

            def drain(n):
                for _ in range(min(n, len(filler))):
                    filler.popleft()()

            def drain_all():
                while filler:
                    filler.popleft()()

            states = [None] * B
            states[0] = alloc_batch(0)
            xts = {(0, 0): qkv_dma(0, 0)}
            hm, wfc, biasf, rc = _late_consts()
            # warm up the collective stream so the first real AllToAll does
            # not absorb the cross-core barrier + algorithm setup (~20us)
            if not no_collective:
                w_in = dpool.tile([NCORES, 16], bf16, name="warm_in")
                w_out = dpool.tile([NCORES, 16], bf16, name="warm_out")
                wsb = cst.tile([8, 16], bf16, name="warm_sb")
                nc.gpsimd.memset(wsb[:], 0.0)
                nc.sync.dma_start(w_in[:], wsb[:])
                nc.gpsimd.collective_compute(
                    "AllToAll", mybir.AluOpType.bypass,
                    replica_groups=[list(range(NCORES))],
                    ins=[w_in.opt()], outs=[w_out.opt()])
            for ch in range(NCH_B):
                if ch + 1 < NCH_B:
                    xts[(0, ch + 1)] = qkv_dma(0, ch + 1)
                for u in qkv_units(0, ch, xts.pop((0, ch)), states[0]):
                    u()

            ag_outs = [None] * B
            ndum = 0
            for b in range(B):
                if b + 1 < B:
                    states[b + 1] = alloc_batch(b + 1)
                for qc in range(QC):
                    if b + 1 < B:
                        xts[(b + 1, qc)] = qkv_dma(b + 1, qc)
                        filler.extend(qkv_units(b + 1, qc,
                                                xts.pop((b + 1, qc)),
                                                states[b + 1]))
                    if qc == 2 and b >= 1:
                        filler.extend(fc_units(b - 1, ag_outs[b - 1],
                                               wfc, biasf))
                    if b == B - 1 and qc >= 2:
                        # no next-batch QKV to interleave: keep PE warm
                        for _ in range(6):
                            ndum += 1
                            filler.append(dummy_unit(ndum))
                    attn_qc(b, qc, states[b], hm, rc, drain)
                drain_all()
                normalize_batch(b, states[b], rc)
                ag_outs[b] = a2a_batch(b, states[b])
            # tail: keep the PE warm while AllToAll(3) completes
            for _ in range(14):
                ndum += 1
                dummy_unit(ndum)()
            for u in fc_units(B - 1, ag_outs[B - 1], wfc, biasf):
                u()

    nc.compile()
    return nc


def _host_inputs(x, W_qkv, b_qkv, W_fc, b_fc):
    import ml_dtypes
    bf = ml_dtypes.bfloat16
    x = np.asarray(x, dtype=np.float32)
    W_qkv = np.asarray(W_qkv, dtype=np.float32)
    b_qkv = np.asarray(b_qkv, dtype=np.float32)
    W_fc = np.asarray(W_fc, dtype=np.float32)
    b_fc = np.asarray(b_fc, dtype=np.float32)

    xT = np.ascontiguousarray(x.reshape(BT, D).T).astype(bf)
    hm = (np.arange(128)[:, None]
          <= np.arange(896)[None, :] - 384).astype(bf)
    # full FC weights prepacked to SBUF layout [p, (f*8+d)*128 + c]
    wfc = np.ascontiguousarray(
        W_fc.reshape(8, 128, 8, 128).transpose(1, 2, 0, 3).reshape(128, 8192)
    ).astype(bf)
    bfc = np.ascontiguousarray(b_fc[None, :]).astype(bf)
    in_maps = []
    for c in range(NCORES):
        f0 = c * (HPC * HD)  # 128*c
        wqs = np.concatenate(
            [W_qkv[:, p * D + f0: p * D + f0 + 128] for p in range(3)],
            axis=1)  # [1024, 384] = [q|k|v]
        wq_c = np.ascontiguousarray(
            wqs.reshape(8, 128, 384).transpose(1, 0, 2).reshape(128, 3072)
        ).astype(bf)
        bq_c = np.ascontiguousarray(np.concatenate(
            [b_qkv[p * D + f0: p * D + f0 + 128] for p in range(3)])[None, :]
        ).astype(bf)
        in_maps.append({
            "xT": xT, "wq": wq_c, "bq": bq_c, "wfc": wfc, "bfc": bfc,
            "hm": hm,
        })
    return in_maps


def _get_nc():
    if "nc" not in _CACHE:
        _CACHE["nc"] = _build()
    return _CACHE["nc"]


def _assemble(results):
    full = np.empty((BT, D), dtype=np.float32)
    for c in range(NCORES):
        o = results[c]["outT"]  # [1024 features, 4*256 tokens]
        for b in range(B):
            full[b * T + c * TOK: b * T + (c + 1) * TOK, :] = \
                o[:, b * TOK:(b + 1) * TOK].T
    return full.reshape(B, T, D)


def kernel(x, W_qkv, b_qkv, W_fc, b_fc):
    nc = _get_nc()
    in_maps = _host_inputs(x, W_qkv, b_qkv, W_fc, b_fc)
    res = run_bass_kernel_spmd(nc, in_maps, list(range(NCORES)))
    return _assemble(res.results)


# revision 15
# speedup vs baseline: 1.6274x; 1.0012x over previous
"""Trainium2 Bass kernel for causal multi-head attention (B=4, T=2048, D=1024, H=16).

Sharding: tensor-parallel over heads for QKV+attention (each of 8 cores owns
2 heads over all tokens), then AllToAll re-shards from head-sharded to
token-sharded so each core computes the final FC over the full feature dim
for its 256-token slice of each batch.

All matmuls run in bf16 (fp32 streams at half PE rate; bf16 at full), with
fp32 PSUM accumulation. Scores are computed transposed (S^T = K Q^T, two
heads packed in PE quadrants via tile_position) so softmax normalization
lands on the PV matmul's free dim; the denominator comes from a ones column
augmented into V. Normalization is deferred out of the inner loop (the
per-chunk broadcast chain caused PE bubbles + HAM re-throttling). The
denominator reciprocal runs on the scalar engine (DVE reciprocal is 3.3us
for [1,512] and stalls the next chunk via a PSUM WAR; custom-DVE
reciprocal_approx_fast and AluOp divide are broken/unsupported on this
stack), and the broadcast across partitions uses a selector matmul
(partition-shifted DVE/DMA ops misbehave; SBUF-side DMA access patterns
must keep the partition dim outermost). Mask-multiplies and normalize
multiplies run on the otherwise-idle GPSIMD engine (it cannot read PSUM).
One AllToAll per batch (smaller per-half collectives measured ~3x slower
per byte), and QKV of batch b+1 / FC of batch b-1 interleave into batch
b's attention to keep the PE fed while the scalar engine runs exp.
"""
import sys

for _p in ("/opt/trn_rl_repo",):
    if _p not in sys.path:
        sys.path.insert(0, _p)

import numpy as np

import concourse.bass as bass
import concourse.mybir as mybir
import concourse.tile as tile
from concourse import bacc
from concourse.bass_utils import run_bass_kernel_spmd

f32 = mybir.dt.float32
bf16 = mybir.dt.bfloat16
EXP = mybir.ActivationFunctionType.Exp

B, T, D, H, HD = 4, 2048, 1024, 16, 64
NCORES = 8
HPC = H // NCORES          # heads per core = 2
BT = B * T                 # 8192
CH = 512                   # token chunk (q chunk / projection chunk)
NCH_B = T // CH            # 4 projection chunks per batch
QC = T // CH               # 4 query chunks per batch
NKV_B = T // 128           # 16 kv tiles of 128 per batch
TOK = 256                  # tokens per core per batch (after AllToAll)
SCALE = 1.0 / 8.0          # 1/sqrt(HD)

RECIP_MODE = "scalar"

_CACHE = {}


def _build(no_collective=False):
    nc = bacc.Bacc("TRN2", target_bir_lowering=False, debug=False,
                   num_devices=NCORES)

    xT = nc.dram_tensor("xT", [D, BT], bf16, kind="ExternalInput").ap()
    wq_d = nc.dram_tensor("wq", [128, 8 * 384], bf16, kind="ExternalInput").ap()
    bq_d = nc.dram_tensor("bq", [1, 3 * 128], bf16, kind="ExternalInput").ap()
    wfc_d = nc.dram_tensor("wfc", [128, 64 * 128], bf16,
                           kind="ExternalInput").ap()
    bfc_d = nc.dram_tensor("bfc", [1, D], bf16, kind="ExternalInput").ap()
    hm_d = nc.dram_tensor("hm", [128, 896], bf16, kind="ExternalInput").ap()
    outT = nc.dram_tensor("outT", [D, B * TOK], f32,
                          kind="ExternalOutput").ap()

    with tile.TileContext(nc) as tc:
        with tc.tile_pool(name="const", bufs=1) as cst, \
             tc.tile_pool(name="dram", bufs=1, space="DRAM") as dpool, \
             tc.tile_pool(name="work", bufs=1) as wk, \
             tc.tile_pool(name="ps", bufs=1, space="PSUM") as ps:

            # ---- weights needed first (QKV of batch 0) ----
            wq = cst.tile([128, 8 * 384], bf16)
            nc.sync.dma_start(wq[:], wq_d[:])
            biasq = cst.tile([1, 3 * 128], bf16)
            nc.sync.dma_start(biasq[:], bq_d[:])
            onesb = cst.tile([1, CH], bf16)
            nc.gpsimd.memset(onesb[:], 1.0)
            # selector for reciprocal broadcast: row 64 = 1, rest 0
            zl = cst.tile([65, 64], bf16)
            nc.gpsimd.memset(zl[:], 0.0)
            nc.gpsimd.memset(zl[64:65, :], 1.0)

            # ---- deferred constants (needed later; don't block first mm) ----
            def _late_consts():
                hm = cst.tile([128, 896], bf16)
                nc.sync.dma_start(hm[:], hm_d[:])
                wfc = cst.tile([128, 64 * 128], bf16)
                nc.sync.dma_start(wfc[:], wfc_d[:])
                biasf = cst.tile([1, D], bf16)
                nc.sync.dma_start(biasf[:], bfc_d[:])
                # per-head reciprocal staging: row 64 = recip, rows 0-63
                # zeroed once (garbage would poison the selector matmul)
                rc = []
                for h in range(HPC):
                    t = cst.tile([65, T], bf16, name=f"rc{h}")
                    nc.gpsimd.memset(t[0:64, :], 0.0)
                    rc.append(t)
                return hm, wfc, biasf, rc

            # ---- per-batch state (double buffered across the pipeline) ----
            def alloc_batch(b):
                qt = wk.tile([128, T], bf16, tag="qt", bufs=2, name=f"qt{b}")
                kt = wk.tile([128, T], bf16, tag="kt", bufs=2, name=f"kt{b}")
                vsb = wk.tile([128, NKV_B * 130], bf16, tag="vsb", bufs=2,
                              name=f"vsb{b}")
                v3 = vsb.rearrange("p (t c) -> p t c", c=130)
                nc.gpsimd.memset(v3[:, :, 64:65], 1.0)
                nc.gpsimd.memset(v3[:, :, 129:130], 1.0)
                praw = [wk.tile([64, T], bf16, tag=f"praw{h}", bufs=2,
                                name=f"praw{h}_{b}") for h in range(HPC)]
                return qt, kt, vsb, praw

            def qkv_dma(b, ch):
                """Prefetch one 512-token x chunk."""
                c0 = b * T + ch * CH
                xt = wk.tile([128, 8 * CH], bf16, tag="xt", bufs=3,
                             name=f"xt{b}_{ch}")
                xt3 = xt.rearrange("p (d c) -> p d c", d=8)
                xs3 = xT[:, c0:c0 + CH].rearrange("(d p) c -> p d c", p=128)
                nc.sync.dma_start(xt3[:], xs3)
                return xt

            def qkv_units(b, ch, xt, st):
                """Chunk projection as self-contained PE units (aux PSUM).

                Each unit allocates its own short-lived [128,512] aux tile so
                units can interleave with attention without pinning the
                st-tag rotation."""
                qt, kt, vsb = st[0], st[1], st[2]
                cs = ch * CH
                xt3 = xt.rearrange("p (d c) -> p d c", d=8)

                def q_unit(tgt, wo, bo):
                    def emit():
                        pq = ps.tile([128, CH], f32, tag="aux", bufs=2,
                                     name=f"pq{b}_{ch}_{wo}")
                        for d in range(8):
                            nc.tensor.matmul(pq[:],
                                             wq[:, d * 384 + wo:d * 384 + wo + 128],
                                             xt[:, d * CH:(d + 1) * CH],
                                             start=(d == 0), stop=False)
                        nc.tensor.matmul(pq[:], biasq[0:1, bo:bo + 128],
                                         onesb[0:1, :], start=False, stop=True)
                        nc.vector.tensor_copy(tgt[:, cs:cs + CH], pq[:])
                    return emit

                def v_unit(sb):
                    def emit():
                        kvt = ch * 4 + sb
                        pvv = ps.tile([128, CH], f32, tag="aux", bufs=2,
                                      name=f"pvv{b}_{ch}_{sb}")
                        for d in range(8):
                            nc.tensor.matmul(
                                pvv[:, 0:128],
                                xt3[:, d, sb * 128:(sb + 1) * 128],
                                wq[:, d * 384 + 256:d * 384 + 384],
                                start=(d == 0), stop=False)
                        nc.tensor.matmul(pvv[:, 0:128], onesb[0:1, 0:128],
                                         biasq[0:1, 256:384],
                                         start=False, stop=True)
                        base = kvt * 130
                        nc.vector.tensor_copy(vsb[:, base:base + 64],
                                              pvv[:, 0:64])
                        nc.vector.tensor_copy(vsb[:, base + 65:base + 129],
                                              pvv[:, 64:128])
                    return emit

                return [q_unit(qt, 0, 0), q_unit(kt, 128, 128),
                        v_unit(0), v_unit(1), v_unit(2), v_unit(3)]

            def _recip(out, in_):
                if RECIP_MODE == "scalar":
                    # ScalarE table-based reciprocal: ~0.7us for [1,512] vs
                    # 3.3us on DVE; accuracy (~1e-3) is plenty for the bf16
                    # broadcast that consumes it.
                    imm = mybir.ImmediateValue
                    nc.scalar.add_instruction(
                        mybir.InstActivation(
                            name=nc.get_next_instruction_name(),
                            func=mybir.ActivationFunctionType.Reciprocal,
                            ins=[nc.scalar.lower_ap(in_),
                                 imm(dtype=mybir.dt.float32, value=0.0),
                                 imm(dtype=mybir.dt.float32, value=1.0),
                                 imm(dtype=mybir.dt.float32, value=0.0)],
                            outs=[nc.scalar.lower_ap(out)],
                        ))
                else:
                    with nc.allow_low_precision(reason="bf16 denom recip"):
                        nc.vector.reciprocal(out, in_)

            def attn_qc(b, qc, st, hm, rc, drain):
                """Causal attention for query chunk qc of batch b."""
                qt, kt, vsb, praw = st
                nkv = 4 * (qc + 1)
                pv = [ps.tile([65, CH], f32, tag=f"pv{h}", bufs=1,
                              name=f"pv{h}_{b}_{qc}") for h in range(HPC)]
                for ki in range(nkv):
                    diag = ki - 4 * qc
                    # on diagonal tiles, queries below the block are fully
                    # masked: shrink the moving dim to the causal range
                    off = 128 * diag if diag > 0 else 0
                    m = CH - off
                    stt = ps.tile([128, 2 * CH], f32, tag="st", bufs=2,
                                  name=f"s_{b}_{qc}_{ki}")
                    pt = wk.tile([128, 2 * CH], bf16, tag="pt", bufs=3,
                                 name=f"p_{b}_{qc}_{ki}")
                    for h in range(HPC):
                        nc.tensor.matmul(
                            stt[:, h * CH + off:(h + 1) * CH],
                            kt[64 * h:64 * h + 64, ki * 128:(ki + 1) * 128],
                            qt[64 * h:64 * h + 64,
                               qc * CH + off:(qc + 1) * CH],
                            start=True, stop=True,
                            tile_position=(64 * h, 0))
                    # fill the scores->exp->PV latency with independent PE
                    # work: the unit lands between S(ki) and PV(ki) in the
                    # in-order PE queue, so PV no longer stalls on exp
                    drain(1)
                    if diag > 0:
                        for h in range(HPC):
                            nc.scalar.activation(
                                pt[:, h * CH + off:(h + 1) * CH],
                                stt[:, h * CH + off:(h + 1) * CH],
                                EXP, scale=SCALE)
                    else:
                        nc.scalar.activation(pt[:], stt[:], EXP, scale=SCALE)
                    if diag >= 0:
                        for h in range(HPC):
                            nc.gpsimd.tensor_mul(
                                pt[:, h * CH + off:(h + 1) * CH],
                                pt[:, h * CH + off:(h + 1) * CH],
                                hm[:, 384:384 + m])
                    for h in range(HPC):
                        vb = ki * 130 + 65 * h
                        nc.tensor.matmul(pv[h][0:65, off:CH],
                                         vsb[:, vb:vb + 65],
                                         pt[:, h * CH + off:(h + 1) * CH],
                                         start=(ki == 0),
                                         stop=(ki == nkv - 1))
                # stash raw PV + reciprocal of the denominator row
                for h in range(HPC):
                    nc.vector.tensor_copy(praw[h][:, qc * CH:(qc + 1) * CH],
                                          pv[h][0:64, :])
                    _recip(rc[h][64:65, qc * CH:(qc + 1) * CH],
                           pv[h][64:65, :])

            def normalize_batch(b, st, rc):
                """praw *= broadcast(1/denom) via selector matmul."""
                praw = st[3]
                for qc in range(QC):
                    for h in range(HPC):
                        bcq = ps.tile([128, CH], f32, tag="aux", bufs=2,
                                      name=f"bc{h}_{b}_{qc}")
                        nc.tensor.matmul(bcq[0:64, :], zl[:, 0:64],
                                         rc[h][:, qc * CH:(qc + 1) * CH],
                                         start=True, stop=True)
                        rbs = wk.tile([64, CH], bf16, tag="rbs", bufs=2,
                                      name=f"rbs{h}_{b}_{qc}")
                        nc.vector.tensor_copy(rbs[:], bcq[0:64, :])
                        nc.gpsimd.tensor_mul(
                            praw[h][:, qc * CH:(qc + 1) * CH],
                            praw[h][:, qc * CH:(qc + 1) * CH], rbs[:])

            def a2a_batch(b, st):
                """Re-shard batch b attention output: head- to token-sharded."""
                praw = st[3]
                ag_in = dpool.tile([NCORES * 128, TOK], bf16,
                                   name=f"ag_in{b}")
                ag_out = dpool.tile([NCORES * 128, TOK], bf16,
                                    name=f"ag_out{b}")
                div = ag_in.rearrange("(d p) c -> p d c", p=128)
                for h in range(HPC):
                    src = praw[h].rearrange("p (d c) -> p d c", c=TOK)
                    nc.sync.dma_start(div[64 * h:64 * h + 64, :, :], src)
                if no_collective:
                    nc.sync.dma_start(ag_out[:], ag_in[:])
                else:
                    nc.gpsimd.collective_compute(
                        "AllToAll", mybir.AluOpType.bypass,
                        replica_groups=[list(range(NCORES))],
                        ins=[ag_in.opt()], outs=[ag_out.opt()])
                return ag_out

            def fc_units(b, ag_out, wfc, biasf):
                """Full FC for this core's 256-token slice, as PE units."""
                box = {}

                def load():
                    fci = wk.tile([128, 8 * TOK], bf16, tag="fci", bufs=2,
                                  name=f"fci{b}")
                    fci3 = fci.rearrange("p (d c) -> p d c", d=8)
                    srcv = ag_out.rearrange("(d p) c -> p d c", p=128)
                    nc.sync.dma_start(fci3[:], srcv)
                    box["fci"] = fci
                    box["ost"] = wk.tile([128, 8 * TOK], f32, tag="ost",
                                         bufs=2, name=f"ost{b}")

                def fo_unit(fo):
                    def emit():
                        fci, ost = box["fci"], box["ost"]
                        pfc = ps.tile([128, CH], f32, tag="aux", bufs=2,
                                      name=f"pfc{b}_{fo}")
                        for d in range(8):
                            nc.tensor.matmul(
                                pfc[:, 0:TOK],
                                wfc[:, (fo * 8 + d) * 128:
                                     (fo * 8 + d + 1) * 128],
                                fci[:, d * TOK:(d + 1) * TOK],
                                start=(d == 0), stop=False)
                        nc.tensor.matmul(pfc[:, 0:TOK],
                                         biasf[0:1, fo * 128:(fo + 1) * 128],
                                         onesb[0:1, 0:TOK],
                                         start=False, stop=True)
                        nc.vector.tensor_copy(
                            ost[:, fo * TOK:(fo + 1) * TOK], pfc[:, 0:TOK])
                    return emit

                def store():
                    dst = outT.rearrange("(f p) c -> p f c", p=128)[
                        :, :, b * TOK:(b + 1) * TOK]
                    osrc = box["ost"].rearrange("p (f c) -> p f c", c=TOK)
                    nc.sync.dma_start(dst, osrc)

                return [load] + [fo_unit(fo) for fo in range(8)] + [store]

            def dummy_unit(tag_i):
                """~0.9us of dependency-free matmuls to keep the HAM warm."""
                def emit():
                    pdm = ps.tile([128, CH], f32, tag="aux", bufs=2,
                                  name=f"pdm{tag_i}")
                    for r in range(4):
                        nc.tensor.matmul(pdm[0:1, :], onesb[0:1, 0:1],
                                         onesb[0:1, :],
                                         start=True, stop=True)
                return emit

            # ================= schedule =================
            from collections import deque
            filler = deque()

            def drain(n):
                for _ in range(min(n, len(filler))):
                    filler.popleft()()

            def drain_all():
                while filler:
                    filler.popleft()()

            states = [None] * B
            states[0] = alloc_batch(0)
            xts = {(0, 0): qkv_dma(0, 0)}
            hm, wfc, biasf, rc = _late_consts()
            # warm up the collective stream so the first real AllToAll does
            # not absorb the cross-core barrier + algorithm setup (~20us)
            if not no_collective:
                w_in = dpool.tile([NCORES, 16], bf16, name="warm_in")
                w_out = dpool.tile([NCORES, 16], bf16, name="warm_out")
                wsb = cst.tile([8, 16], bf16, name="warm_sb")
                nc.gpsimd.memset(wsb[:], 0.0)
                nc.sync.dma_start(w_in[:], wsb[:])
                nc.gpsimd.collective_compute(
                    "AllToAll", mybir.AluOpType.bypass,
                    replica_groups=[list(range(NCORES))],
                    ins=[w_in.opt()], outs=[w_out.opt()])
            for ch in range(NCH_B):
                if ch + 1 < NCH_B:
                    xts[(0, ch + 1)] = qkv_dma(0, ch + 1)
                for u in qkv_units(0, ch, xts.pop((0, ch)), states[0]):
                    u()

            ag_outs = [None] * B
            ndum = 0
            for b in range(B):
                if b + 1 < B:
                    states[b + 1] = alloc_batch(b + 1)
                for qc in range(QC):
                    if b + 1 < B:
                        xts[(b + 1, qc)] = qkv_dma(b + 1, qc)
                        filler.extend(qkv_units(b + 1, qc,
                                                xts.pop((b + 1, qc)),
                                                states[b + 1]))
                    if qc == 2 and b >= 1:
                        filler.extend(fc_units(b - 1, ag_outs[b - 1],
                                               wfc, biasf))
                    if b == B - 1 and qc >= 2:
                        # no next-batch QKV to interleave: keep PE warm
                        for _ in range(6):
                            ndum += 1
                            filler.append(dummy_unit(ndum))
                    attn_qc(b, qc, states[b], hm, rc, drain)
                drain_all()
                normalize_batch(b, states[b], rc)
                ag_outs[b] = a2a_batch(b, states[b])
            # tail: keep the PE warm while AllToAll(3) completes
            for _ in range(14):
                ndum += 1
                dummy_unit(ndum)()
            for u in fc_units(B - 1, ag_outs[B - 1], wfc, biasf):
                u()

    nc.compile()
    return nc


def _host_inputs(x, W_qkv, b_qkv, W_fc, b_fc):
    import ml_dtypes
    bf = ml_dtypes.bfloat16
    x = np.asarray(x, dtype=np.float32)
    W_qkv = np.asarray(W_qkv, dtype=np.float32)
    b_qkv = np.asarray(b_qkv, dtype=np.float32)
    W_fc = np.asarray(W_fc, dtype=np.float32)
    b_fc = np.asarray(b_fc, dtype=np.float32)

    xT = np.ascontiguousarray(x.reshape(BT, D).T).astype(bf)
    hm = (np.arange(128)[:, None]
          <= np.arange(896)[None, :] - 384).astype(bf)
    # full FC weights prepacked to SBUF layout [p, (f*8+d)*128 + c]
    wfc = np.ascontiguousarray(
        W_fc.reshape(8, 128, 8, 128).transpose(1, 2, 0, 3).reshape(128, 8192)
    ).astype(bf)
    bfc = np.ascontiguousarray(b_fc[None, :]).astype(bf)
    in_maps = []
    for c in range(NCORES):
        f0 = c * (HPC * HD)  # 128*c
        wqs = np.concatenate(
            [W_qkv[:, p * D + f0: p * D + f0 + 128] for p in range(3)],
            axis=1)  # [1024, 384] = [q|k|v]
        wq_c = np.ascontiguousarray(
            wqs.reshape(8, 128, 384).transpose(1, 0, 2).reshape(128, 3072)
        ).astype(bf)
        bq_c = np.ascontiguousarray(np.concatenate(
            [b_qkv[p * D + f0: p * D + f0 + 128] for p in range(3)])[None, :]
        ).astype(bf)
        in_maps.append({
            "xT": xT, "wq": wq_c, "bq": bq_c, "wfc": wfc, "bfc": bfc,
            "hm": hm,
        })
    return in_maps


def _get_nc():
    if "nc" not in _CACHE:
        _CACHE["nc"] = _build()
    return _CACHE["nc"]


def _assemble(results):
    full = np.empty((BT, D), dtype=np.float32)
    for c in range(NCORES):
        o = results[c]["outT"]  # [1024 features, 4*256 tokens]
        for b in range(B):
            full[b * T + c * TOK: b * T + (c + 1) * TOK, :] = \
                o[:, b * TOK:(b + 1) * TOK].T
    return full.reshape(B, T, D)


def kernel(x, W_qkv, b_qkv, W_fc, b_fc):
    nc = _get_nc()
    in_maps = _host_inputs(x, W_qkv, b_qkv, W_fc, b_fc)
    res = run_bass_kernel_spmd(nc, in_maps, list(range(NCORES)))
    return _assemble(res.results)
